# revision 39
# baseline (speedup 1.0000x reference)
"""Trainium2 Bass kernel for nn_CIFARViT: 8-layer ViT with a per-head
mini-transformer over attention maps. Data-parallel: one batch element
per NeuronCore (8 cores), full inputs in / full outputs out.

v2 (engine-balance rewrite):
  - softmax max-subtraction dropped (logits verified |s| < 2).
  - all per-head [128,1] stat ops batched into multi-column tiles; one
    reciprocal per stage instead of per (head, chunk).
  - rsqrt = Exp(-0.5*Ln(v+eps)) on ACT (natural_log_exp table set, same
    set as the score exps -> no table thrash).
  - LayerNorm stats via bn_stats/bn_aggr.
  - gamma/beta + PSUM->SBUF moves fused into ACT Identity ops; plain
    copies alternate DVE/ACT to balance engine load.
  - mini-transformer restructured into 4-head stage-batched groups.
  - per-head K=96 proj (kills the 24-way oT copy split).
  - wn column-sum reciprocals: transpose Z rows to columns on PE first,
    then ONE [128,16] reciprocal (was 64 serial [1,256] recips).
"""
import sys

sys.path.insert(0, "/opt/trn_rl_repo")

import numpy as np
import ml_dtypes

import concourse.bass as bass
import concourse.mybir as mybir
import concourse.tile as tile
from concourse import bacc
from concourse.bass_utils import run_bass_kernel_spmd

BF = ml_dtypes.bfloat16
F32 = np.float32
AF = mybir.ActivationFunctionType
ALU = mybir.AluOpType
bf = mybir.dt.bfloat16
f32 = mybir.dt.float32

H = 8
HD = 96
C = 768
L = 256
D_LAYERS = 8
F = 3072

N_CORES = 8
X_AXIS = mybir.AxisListType.X
import os
DBG_LAYERS = int(os.environ.get("KLAYERS", "8"))
DBG_TAP = os.environ.get("KTAP", "") == "1"
DBG_EPI = os.environ.get("KEPI", "1") == "1"


def _fold(wt):
    """[R, Cc] with R = 128*T -> [128, T*Cc] partition fold."""
    R, Cc = wt.shape
    T = R // 128
    return np.ascontiguousarray(
        wt.reshape(T, 128, Cc).transpose(1, 0, 2).reshape(128, T * Cc)
    )


def _foldv(v):
    T = v.shape[0] // 128
    return np.ascontiguousarray(v.reshape(T, 128).T)


def marshal(inputs):
    inp = {k: np.asarray(v) for k, v in inputs.items()}
    sh = {}
    qk_l, v_l, proj_l, w1_l, w2_l, lnp_l, b1_l, rowb_l = ([] for _ in range(8))
    for i in range(D_LAYERS):
        qkvT = inp["qkv_w"][i].T.astype(BF)  # [768, 2304]
        qkf = _fold(qkvT)  # [128, 6*2304]
        img = np.zeros((128, 4 * 2304), dtype=BF)
        for hp in range(4):
            for cb in range(6):
                for hh in range(2):
                    h = hp * 2 + hh
                    base = hp * 2304 + cb * 384 + hh * 192
                    img[:, base:base + 96] = \
                        qkf[:, cb * 2304 + 96 * h: cb * 2304 + 96 * h + 96]
                    img[:, base + 96:base + 192] = \
                        qkf[:, cb * 2304 + 768 + 96 * h: cb * 2304 + 768 + 96 * h + 96]
        qk_l.append(img)
        v_l.append(_fold(np.ascontiguousarray(qkvT[:, 1536:2304])))
        # per-head proj rows: [96, 8*768]  (head h rows 96h..96h+96 of W^T)
        pw = inp["proj_w"][i].T.astype(BF)  # [768, 768]
        ph = np.zeros((96, 8 * 768), dtype=BF)
        for h in range(8):
            ph[:, h * 768:(h + 1) * 768] = pw[96 * h:96 * h + 96, :]
        proj_l.append(ph)
        w1T = inp["mlp_w1"][i].T.astype(BF)  # [768, 3072]
        w1_l.append(np.ascontiguousarray(
            w1T.reshape(6, 128, 24, 128).transpose(1, 2, 0, 3).reshape(128, 24 * 768)))
        w2_l.append(_fold(inp["mlp_w2"][i].T.astype(BF)))  # [128, 24*768]
        lnp_l.append(np.concatenate(
            [_foldv(inp[k][i].astype(F32))
             for k in ("ln1_g", "ln1_b", "ln2_g", "ln2_b")], axis=1))
        b1_l.append(_foldv(inp["mlp_b1"][i].astype(F32)))
        rb = np.zeros((2, 2 * C), dtype=BF)
        rb[0, :C] = inp["proj_b"][i].astype(BF)
        rb[0, C:] = inp["mlp_b2"][i].astype(BF)
        rowb_l.append(rb)
    sh["qk_img"] = np.stack(qk_l)
    sh["v_img"] = np.stack(v_l)
    sh["projT"] = np.stack(proj_l)
    sh["w1T"] = np.stack(w1_l)
    sh["w2T"] = np.stack(w2_l)
    sh["lnp"] = np.stack(lnp_l)
    sh["b1f"] = np.stack(b1_l)
    sh["rowb"] = np.stack(rowb_l)

    sh["mqkvT"] = _fold(inp["m_qkv_w"].T.astype(BF))   # [128, 2*768]
    sh["mprojT"] = _fold(inp["m_proj_w"].T.astype(BF))  # [128, 2*256]
    sh["mw1T"] = _fold(inp["m_mlp_w1"].T.astype(BF))   # [128, 2*1024]
    sh["mw2T"] = _fold(inp["m_mlp_w2"].T.astype(BF))   # [128, 8*256]
    sh["mlnp"] = np.concatenate(
        [_foldv(inp[k].astype(F32))
         for k in ("m_ln1_g", "m_ln1_b", "m_ln2_g", "m_ln2_b")], axis=1)
    sh["mb1f"] = _foldv(inp["m_mlp_b1"].astype(F32))   # [128, 8]
    mrb_ = np.zeros((2, 512), dtype=BF)
    mrb_[0, :256] = inp["m_proj_b"].astype(BF)
    mrb_[0, 256:] = inp["m_mlp_b2"].astype(BF)
    sh["mrowb"] = mrb_

    sh["pwT"] = np.ascontiguousarray(inp["patch_w"].reshape(C, 12).T.astype(BF))
    pos = inp["pos_emb"][0].astype(F32) + inp["patch_b"][None, :].astype(F32)
    sh["pos"] = _fold(pos)  # [128, 2*768]
    sh["normgb"] = np.concatenate(
        [_foldv(inp["norm_g"].astype(F32)), _foldv(inp["norm_b"].astype(F32))],
        axis=1)  # [128, 12]
    sh["headwT"] = _fold(inp["head_w"].T.astype(F32))  # [128, 6*10]
    sh["headb"] = inp["head_b"].astype(F32).reshape(10, 1)
    sh["ident"] = np.eye(128, dtype=BF)
    sh["onescol"] = np.ones((128, 1), dtype=BF)
    sh["ones2"] = np.ones((2, 128), dtype=BF)
    e0_ = np.zeros((2, 1), dtype=BF); e0_[0, 0] = 1.0
    sh["e0"] = e0_

    x = inp["x"].astype(F32)
    per_core = []
    for b in range(N_CORES):
        pt = (x[b].reshape(3, 16, 2, 16, 2).transpose(0, 2, 4, 1, 3)
              .reshape(12, 256).astype(BF))
        m = dict(sh)
        m["patchesT"] = np.ascontiguousarray(pt)
        per_core.append(m)
    return per_core


DT_MAP = {np.dtype(BF): bf, np.dtype(np.float32): f32}


def build(in_map):
    nc = bacc.Bacc("TRN2", target_bir_lowering=False, debug=False,
                   num_devices=N_CORES)
    dram = {k: nc.dram_tensor(k, v.shape, DT_MAP[v.dtype], kind="ExternalInput")
            for k, v in in_map.items()}
    out_d = nc.dram_tensor("out", (10, 1), f32, kind="ExternalOutput")
    dbg_d = nc.dram_tensor("dbg", (128, 10 * C), f32,
                           kind="ExternalOutput") if DBG_TAP else None
    with tile.TileContext(nc) as tc:
        _body(nc, tc, dram, out_d, dbg_d)
    nc.compile()
    return nc


def _body(nc, tc, dram, out_d, dbg_d=None):
    import contextlib
    ctx = contextlib.ExitStack()
    with ctx:
        P = lambda name, bufs=1, space="SBUF": ctx.enter_context(
            tc.tile_pool(name=name, bufs=bufs, space=space))
        cpool = P("const")

        def cload(name):
            arr = dram[name]
            t = cpool.tile(list(arr.shape), arr.dtype, tag=name, name=name)
            nc.sync.dma_start(t[:], arr[:])
            return t

        mqkvT = cload("mqkvT")
        mprojT = cload("mprojT")
        mw1T = cload("mw1T")
        mw2T = cload("mw2T")
        mlnp = cload("mlnp")
        mb1f = cload("mb1f")
        mrowb = cload("mrowb")
        ident = cload("ident")
        onescol = cload("onescol")
        ones2 = cload("ones2")
        e0 = cload("e0")
        normgb = cload("normgb")
        headwT = cload("headwT")
        headb = cload("headb")
        pwT = cload("pwT")
        patchesT = cload("patchesT")

        NDIAG = 3
        dtiles = []
        for j in range(NDIAG):
            t = cpool.tile([128, 512], bf, tag=f"diag{j}", name=f"diag{j}")
            nc.gpsimd.memset(t[:], 0.0)
            dtiles.append(t)
        dctr = [0]

        eps6 = cpool.tile([128, 1], f32, tag="eps6", name="eps6")
        nc.gpsimd.memset(eps6[:], 1e-6)
        eps5 = cpool.tile([128, 1], f32, tag="eps5", name="eps5")
        nc.gpsimd.memset(eps5[:], 1e-5)

        def build_diag(s0, s1):
            d = dtiles[dctr[0] % NDIAG]
            dctr[0] += 1
            nc.vector.tensor_scalar_mul(d[:, 0:128], ident[:], s0)
            nc.vector.tensor_scalar_mul(d[:, 384:512], ident[:], s1)
            return d

        # alternate PSUM->SBUF moves between DVE and ACT
        cctr = [0]

        def pcopy(dst, src):
            cctr[0] += 1
            if cctr[0] % 3 != 0:
                nc.vector.tensor_copy(dst, src)
            else:
                nc.scalar.activation(dst, src, AF.Copy)

        hpool = P("h", bufs=3)
        ps = P("ps", bufs=4, space="PSUM")
        psy = P("psy", bufs=4, space="PSUM")
        stp = P("st", bufs=8)
        bsp = P("bst", bufs=16)
        wqk = P("wqk", bufs=2)
        wv = P("wv", bufs=1)
        wproj = P("wproj", bufs=1)
        ww1 = P("ww1", bufs=2)
        ww2 = P("ww2", bufs=2)
        hlnp = P("hln", bufs=1)
        qkp = P("qkt", bufs=1)
        vtp = P("vt", bufs=1)
        ebp = P("eb", bufs=1)
        atp = P("at", bufs=1)
        ybp = P("yb", bufs=1)
        xbp = P("xb", bufs=1)
        scr = P("scr", bufs=2)
        scrq = P("scrq", bufs=3)
        sqp = P("sqp", bufs=2)
        gp = P("gp", bufs=1)
        y1p = P("y1", bufs=3)
        zp = P("zp", bufs=2)
        zrp = P("zrp", bufs=1)
        rbp = P("rbp", bufs=1)

        def bst(w=8):
            return bsp.tile([128, w], f32, tag=f"bst{w}", name="bst")

        def mm(out, lhsT, rhs, start, stop):
            nc.tensor.matmul(out, lhsT, rhs, start=start, stop=stop)

        # ---------------- prologue: patch embed ----------------
        h_t = hpool.tile([128, 2 * C], f32, tag="h", name="h")
        pos_t = cpool.tile([128, 2 * C], f32, tag="pos", name="pos")
        nc.sync.dma_start(pos_t[:], dram["pos"][:])
        for lc in range(2):
            for n0, nw in ((0, 512), (512, 256)):
                p = ps.tile([128, 512], f32, tag="ps", name="ps")
                mm(p[:, 0:nw], patchesT[0:12, lc * 128:lc * 128 + 128],
                   pwT[0:12, n0:n0 + nw], True, True)
                nc.vector.tensor_add(
                    h_t[:, lc * C + n0:lc * C + n0 + nw], p[:, 0:nw],
                    pos_t[:, lc * C + n0:lc * C + n0 + nw])

        # ---------------- layers ----------------
        for li in range(DBG_LAYERS):
            lnp_t = scr.tile([128, 24], f32, tag="lnp", name="lnp")
            nc.sync.dma_start(lnp_t[:], dram["lnp"][li])
            b1f_t = scr.tile([128, 24], f32, tag="b1f", name="b1f")
            nc.sync.dma_start(b1f_t[:], dram["b1f"][li])
            rowb_t = rbp.tile([2, 2 * C], bf, tag="rowb", name="rowb")
            nc.sync.dma_start(rowb_t[:], dram["rowb"][li])
            vt_w = wv.tile([128, 6 * C], bf, tag="wv", name="wv")
            nc.sync.dma_start(vt_w[:], dram["v_img"][li])
            projT = wproj.tile([96, 8 * C], bf, tag="wproj", name="wproj")
            nc.sync.dma_start(projT[:], dram["projT"][li])

            def ln_transpose_outer(src, g_col, b_col, epsv, tag, pool):
                """TM f32 [128, 2*768] -> LN'd FM bf16 [128, 6*256].
                bn_stats stats + rsqrt via gpsimd pow; gamma/beta fused
                into the PSUM->SBUF move on ACT."""
                dst = pool.tile([128, 6 * 256], bf, tag=tag)
                subs = scr.tile([128, 2 * C], bf, tag="lnsub", name="lnsub")
                sdv = bst(2)
                rs = bst(2)
                for lc in range(2):
                    sl = src[:, lc * C:(lc + 1) * C]
                    st3 = scrq.tile([128, 3, 6], f32, tag="st3", name="st3")
                    for sg in range(3):
                        nc.vector.bn_stats(
                            st3[:, sg, :], sl[:, sg * 256:(sg + 1) * 256])
                    mv2 = bsp.tile([128, 2], f32, tag="mv2", name="mv2")
                    nc.vector.bn_aggr(mv2[:, :], st3[:, :, :])
                    nc.scalar.activation(sdv[:, lc:lc + 1], mv2[:, 1:2],
                                         AF.Sqrt, bias=epsv[:])
                    nc.vector.tensor_scalar_sub(
                        subs[:, lc * C:(lc + 1) * C], sl, mv2[:, 0:1])
                nc.vector.reciprocal(rs[:], sdv[:])
                d = build_diag(rs[:, 0:1], rs[:, 1:2])
                for cb in range(6):
                    p = ps.tile([128, 512], f32, tag="ps", name="ps")
                    for lc in range(2):
                        mm(p[:, lc * 128:lc * 128 + 128],
                           subs[:, lc * C + cb * 128:lc * C + cb * 128 + 128],
                           d[:, lc * 384:lc * 384 + 128], True, True)
                    nc.vector.tensor_scalar(
                        dst[:, cb * 256:(cb + 1) * 256], p[:, 0:256],
                        lnp_t[:, g_col + cb:g_col + cb + 1],
                        lnp_t[:, b_col + cb:b_col + cb + 1],
                        ALU.mult, ALU.add)
                return dst

            hln = ln_transpose_outer(h_t, 0, 6, eps6, "hln", hlnp)

            # ---- qkv: q^T|k^T per head [96, 512]; v token-major ----
            qk_bf = qkp.tile([128, 4096], bf, tag="qk", name="qk")
            for hp in range(4):
                qkw = wqk.tile([128, 2304], bf, tag="wqk", name="wqk")
                nc.sync.dma_start(
                    qkw[:], dram["qk_img"][li, :, hp * 2304:(hp + 1) * 2304])
                for hh in range(2):
                    h = hp * 2 + hh
                    p = ps.tile([128, 512], f32, tag="ps", name="ps")
                    for half in range(2):
                        for cb in range(6):
                            lh = qkw[:, cb * 384 + hh * 192 + half * 96:
                                     cb * 384 + hh * 192 + half * 96 + 96]
                            mm(p[0:96, half * 256:half * 256 + 256], lh,
                               hln[:, cb * 256:(cb + 1) * 256],
                               cb == 0, cb == 5)
                    pcopy(qk_bf[0:96, h * 512:(h + 1) * 512], p[0:96, :])

            # ---- attention scores -> E (exp, no max-sub) ----
            E_bf = ebp.tile([128, H * 512], bf, tag="eb", name="eb")
            izz = zp.tile([128, 32], f32, tag="izz", name="izz")
            for h in range(H):
                p = ps.tile([128, 512], f32, tag="ps", name="ps")
                for lc in range(2):
                    mm(p[:, lc * 256:lc * 256 + 256],
                       qk_bf[0:96, h * 512 + lc * 128:h * 512 + lc * 128 + 128],
                       qk_bf[0:96, h * 512 + 256:h * 512 + 512], True, True)
                for lc in range(2):
                    nc.scalar.activation(
                        E_bf[:, h * 512 + lc * 256:h * 512 + lc * 256 + 256],
                        p[:, lc * 256:lc * 256 + 256], AF.Exp,
                        scale=HD ** -0.5,
                        accum_out=izz[:, h * 2 + lc:h * 2 + lc + 1])
            nc.vector.reciprocal(izz[:, 16:32], izz[:, 0:16])

            # ---- v token-major (PE busy while recip runs) ----
            vt_bf = vtp.tile([128, 2 * C], bf, tag="vt", name="vt")
            for lc in range(2):
                for n0, nw in ((0, 512), (512, 256)):
                    p = ps.tile([128, 512], f32, tag="ps", name="ps")
                    for cb in range(6):
                        mm(p[:, 0:nw],
                           hln[:, cb * 256 + lc * 128:cb * 256 + lc * 128 + 128],
                           vt_w[:, cb * C + n0:cb * C + n0 + nw],
                           cb == 0, cb == 5)
                    pcopy(vt_bf[:, lc * C + n0:lc * C + n0 + nw], p[:, 0:nw])

            # ---- aT = (E/Z)^T via transpose-with-diag ----
            aT_bf = atp.tile([128, H * 512], bf, tag="at", name="at")
            for h in range(H):
                d = build_diag(izz[:, 16 + 2 * h:17 + 2 * h],
                               izz[:, 17 + 2 * h:18 + 2 * h])
                pa = ps.tile([128, 512], f32, tag="ps", name="ps")
                for mc in range(2):
                    for lc in range(2):
                        mm(pa[:, mc * 256 + lc * 128:mc * 256 + lc * 128 + 128],
                           E_bf[:, h * 512 + lc * 256 + mc * 128:
                                h * 512 + lc * 256 + mc * 128 + 128],
                           d[:, lc * 384:lc * 384 + 128], True, True)
                pcopy(aT_bf[:, h * 512:(h + 1) * 512], pa[:])

            # ---- attnV -> oT ([96, 8*256], head h at cols h*256) ----
            oT_bf = qkp.tile([128, 4096], bf, tag="qk", name="ot")
            for hq in range(4):
                po = ps.tile([128, 512], f32, tag="ps", name="ps")
                for hh in range(2):
                    h = hq * 2 + hh
                    for mc in range(2):
                        mm(po[0:96, hh * 256:hh * 256 + 256],
                           vt_bf[:, mc * C + 96 * h:mc * C + 96 * h + 96],
                           aT_bf[:, h * 512 + mc * 256:h * 512 + mc * 256 + 256],
                           mc == 0, mc == 1)
                pcopy(oT_bf[0:96, hq * 512:(hq + 1) * 512], po[0:96, :])

            # ---- proj -> y (TM bf16), per-head K=96 accumulation ----
            y_bf = ybp.tile([128, 2 * C], bf, tag="yb", name="yb")
            for lc in range(2):
                for n0, nw in ((0, 512), (512, 256)):
                    p = ps.tile([128, 512], f32, tag="ps", name="ps")
                    for h in range(H):
                        mm(p[:, 0:nw],
                           oT_bf[0:96, h * 256 + lc * 128:h * 256 + lc * 128 + 128],
                           projT[0:96, h * C + n0:h * C + n0 + nw],
                           h == 0, h == 7)
                    pcopy(y_bf[:, lc * C + n0:lc * C + n0 + nw], p[:, 0:nw])

            # ---- mini transformer: 2 groups of 4 heads, stage-batched ----
            X_bf = xbp.tile([128, H * 512], bf, tag="xb", name="xb")
            zrowb_t = zrp.tile([2, H * 256], bf, tag="zrowb", name="zrowb")
            nc.gpsimd.memset(zrowb_t[:], 0.0)
            mizz = zp.tile([128, 32], f32, tag="mizz", name="mizz")
            izc_t = zp.tile([128, 16], f32, tag="izc", name="izc")
            for g in range(2):
                h0 = 4 * g
                izg = izz[:, 16 + 8 * g:24 + 8 * g]   # 1/Z outer, 8 cols
                zg = izz[:, 8 * g:8 + 8 * g]          # Z outer
                # -- mini LN1 on a = E/Z: batched stats --
                sqE = bsp.tile([128, 8], f32, tag="sqE", name="sqE")
                for i in range(8):
                    h, lc = h0 + i // 2, i % 2
                    sq = scrq.tile([128, 256], bf, tag="sq", name="sq")
                    esl = E_bf[:, h * 512 + lc * 256:h * 512 + lc * 256 + 256]
                    nc.vector.scalar_tensor_tensor(
                        sq[:], esl, 1.0, esl, ALU.mult, ALU.mult,
                        accum_out=sqE[:, i:i + 1])
                iz2 = bst()
                nc.vector.tensor_tensor(iz2[:], izg, izg, ALU.mult)
                t1 = bst()
                nc.vector.tensor_tensor(t1[:], sqE[:], iz2[:], ALU.mult)
                var = bst()
                nc.vector.tensor_scalar(var[:], t1[:], 1.0 / L, 1.0 / (L * L),
                                        ALU.mult, ALU.subtract)
                sd8 = bst()
                nc.scalar.activation(sd8[:], var[:], AF.Sqrt, bias=eps6[:])
                ra = bst()
                nc.vector.reciprocal(ra[:], sd8[:])
                rz = bst()
                nc.vector.tensor_tensor(rz[:], ra[:], izg, ALU.mult)
                zc = bst()
                nc.vector.tensor_scalar_mul(zc[:], zg, 1.0 / L)
                esub = gp.tile([128, 2048], bf, tag="esub", name="esub")
                for i in range(8):
                    h, lc = h0 + i // 2, i % 2
                    nc.vector.tensor_scalar_sub(
                        esub[:, i * 256:(i + 1) * 256],
                        E_bf[:, h * 512 + lc * 256:h * 512 + lc * 256 + 256],
                        zc[:, i:i + 1])
                wnlnT = gp.tile([128, 2048], bf, tag="wnln", name="wnln")
                for j in range(4):
                    d = build_diag(rz[:, 2 * j:2 * j + 1], rz[:, 2 * j + 1:2 * j + 2])
                    pw_ = ps.tile([128, 512], f32, tag="ps", name="ps")
                    for mc in range(2):
                        for lc in range(2):
                            mm(pw_[:, mc * 256 + lc * 128:mc * 256 + lc * 128 + 128],
                               esub[:, j * 512 + lc * 256 + mc * 128:
                                    j * 512 + lc * 256 + mc * 128 + 128],
                               d[:, lc * 384:lc * 384 + 128], True, True)
                    for mc in range(2):
                        nc.vector.tensor_scalar(
                            wnlnT[:, j * 512 + mc * 256:j * 512 + mc * 256 + 256],
                            pw_[:, mc * 256:mc * 256 + 256],
                            mlnp[:, 0 + mc:1 + mc], mlnp[:, 2 + mc:3 + mc],
                            ALU.mult, ALU.add)
                # -- mini qkv (q^T|k^T FM per head) --
                mqk = gp.tile([128, 4096], bf, tag="mqk", name="mqk")
                for dc in range(2):
                    pq = [ps.tile([128, 512], f32, tag="ps", name="ps")
                          for _ in range(4)]
                    for half in range(2):
                        for mc in range(2):
                            for j in range(4):
                                mm(pq[j][:, half * 256:half * 256 + 256],
                                   mqkvT[:, mc * 768 + half * 256 + dc * 128:
                                         mc * 768 + half * 256 + dc * 128 + 128],
                                   wnlnT[:, j * 512 + mc * 256:
                                         j * 512 + mc * 256 + 256],
                                   mc == 0, mc == 1)
                    for j in range(4):
                        pcopy(mqk[:, j * 1024 + dc * 512:j * 1024 + dc * 512 + 512],
                              pq[j][:])
                # -- mini v (token-major) --
                mv_bf = gp.tile([128, 2048], bf, tag="mv", name="mv")
                for j in range(4):
                    pv = ps.tile([128, 512], f32, tag="ps", name="ps")
                    for lc in range(2):
                        for mc in range(2):
                            mm(pv[:, lc * 256:lc * 256 + 256],
                               wnlnT[:, j * 512 + mc * 256 + lc * 128:
                                     j * 512 + mc * 256 + lc * 128 + 128],
                               mqkvT[:, mc * 768 + 512:mc * 768 + 768],
                               mc == 0, mc == 1)
                    pcopy(mv_bf[:, j * 512:(j + 1) * 512], pv[:])
                # -- mini scores -> Em = exp (no max-sub) --
                Em = gp.tile([128, 2048], bf, tag="em", name="em")
                for j in range(4):
                    pe = ps.tile([128, 512], f32, tag="ps", name="ps")
                    for lc in range(2):
                        for dc in range(2):
                            mm(pe[:, lc * 256:lc * 256 + 256],
                               mqk[:, j * 1024 + dc * 512 + lc * 128:
                                   j * 1024 + dc * 512 + lc * 128 + 128],
                               mqk[:, j * 1024 + dc * 512 + 256:
                                   j * 1024 + dc * 512 + 512],
                               dc == 0, dc == 1)
                    for lc in range(2):
                        zcol = 8 * g + 2 * j + lc
                        nc.scalar.activation(
                            Em[:, j * 512 + lc * 256:j * 512 + lc * 256 + 256],
                            pe[:, lc * 256:lc * 256 + 256], AF.Exp,
                            scale=0.0625,
                            accum_out=mizz[:, zcol:zcol + 1])
                nc.vector.reciprocal(mizz[:, 16 + 8 * g:24 + 8 * g],
                                     mizz[:, 8 * g:8 + 8 * g])
                # -- amT = (Em/Z)^T --
                amT = gp.tile([128, 2048], bf, tag="mqk", name="amt")
                for j in range(4):
                    zcol = 16 + 8 * g + 2 * j
                    d = build_diag(mizz[:, zcol:zcol + 1],
                                   mizz[:, zcol + 1:zcol + 2])
                    pa = ps.tile([128, 512], f32, tag="ps", name="ps")
                    for mc in range(2):
                        for lc in range(2):
                            mm(pa[:, mc * 256 + lc * 128:mc * 256 + lc * 128 + 128],
                               Em[:, j * 512 + lc * 256 + mc * 128:
                                  j * 512 + lc * 256 + mc * 128 + 128],
                               d[:, lc * 384:lc * 384 + 128], True, True)
                    pcopy(amT[:, j * 512:(j + 1) * 512], pa[:])
                # -- mini attnV -> omT (FM) --
                omT = gp.tile([128, 2048], bf, tag="em", name="omt")
                for j in range(4):
                    po = ps.tile([128, 512], f32, tag="ps", name="ps")
                    for dc in range(2):
                        for mc in range(2):
                            mm(po[:, dc * 256:dc * 256 + 256],
                               mv_bf[:, j * 512 + mc * 256 + dc * 128:
                                     j * 512 + mc * 256 + dc * 128 + 128],
                               amT[:, j * 512 + mc * 256:j * 512 + mc * 256 + 256],
                               mc == 0, mc == 1)
                    pcopy(omT[:, j * 512:(j + 1) * 512], po[:])
                # -- hmini (TM f32) = omT.T@mprojT + aT.T@mprojT + pb --
                hm = gp.tile([128, 2048], f32, tag="hm", name="hm")
                for j in range(4):
                    h = h0 + j
                    for lc in range(2):
                        p = ps.tile([128, 512], f32, tag="ps", name="ps")
                        for mc in range(2):
                            mm(p[:, 0:256],
                               omT[:, j * 512 + mc * 256 + lc * 128:
                                   j * 512 + mc * 256 + lc * 128 + 128],
                               mprojT[:, mc * 256:(mc + 1) * 256],
                               mc == 0, False)
                        for mc in range(2):
                            mm(p[:, 0:256],
                               aT_bf[:, h * 512 + mc * 256 + lc * 128:
                                     h * 512 + mc * 256 + lc * 128 + 128],
                               mprojT[:, mc * 256:(mc + 1) * 256],
                               False, False)
                        mm(p[:, 0:256], ones2[0:2, 0:128],
                           mrowb[0:2, 0:256], False, True)
                        pcopy(hm[:, j * 512 + lc * 256:j * 512 + lc * 256 + 256],
                              p[:, 0:256])
                # -- mini LN2: bn stats, batched rsqrt --
                mvs2 = bsp.tile([128, 16], f32, tag="mvs2", name="mvs2")
                sd2 = bst()
                for i in range(8):
                    st6 = scrq.tile([128, 6], f32, tag="st6", name="st6")
                    nc.vector.bn_stats(
                        st6[:, :], hm[:, i * 256:(i + 1) * 256])
                    nc.vector.bn_aggr(mvs2[:, 2 * i:2 * i + 2], st6[:, :])
                    nc.scalar.activation(sd2[:, i:i + 1],
                                         mvs2[:, 2 * i + 1:2 * i + 2],
                                         AF.Sqrt, bias=eps5[:])
                r2 = bst()
                nc.vector.reciprocal(r2[:], sd2[:])
                hsub = gp.tile([128, 2048], bf, tag="esub", name="hsub")
                for i in range(8):
                    nc.vector.tensor_scalar_sub(
                        hsub[:, i * 256:(i + 1) * 256],
                        hm[:, i * 256:(i + 1) * 256], mvs2[:, 2 * i:2 * i + 1])
                mhln = gp.tile([128, 2048], bf, tag="wnln", name="mhln")
                for j in range(4):
                    d = build_diag(r2[:, 2 * j:2 * j + 1], r2[:, 2 * j + 1:2 * j + 2])
                    ph_ = ps.tile([128, 512], f32, tag="ps", name="ps")
                    for mc in range(2):
                        for lc in range(2):
                            mm(ph_[:, mc * 256 + lc * 128:mc * 256 + lc * 128 + 128],
                               hsub[:, j * 512 + lc * 256 + mc * 128:
                                    j * 512 + lc * 256 + mc * 128 + 128],
                               d[:, lc * 384:lc * 384 + 128], True, True)
                    for mc in range(2):
                        nc.vector.tensor_scalar(
                            mhln[:, j * 512 + mc * 256:j * 512 + mc * 256 + 256],
                            ph_[:, mc * 256:mc * 256 + 256],
                            mlnp[:, 4 + mc:5 + mc], mlnp[:, 6 + mc:7 + mc],
                            ALU.mult, ALU.add)
                # -- mini MLP (4 heads batched per fc; shared LDW) --
                py2 = [psy.tile([128, 512], f32, tag="psy", name="psy")
                       for _ in range(4)]
                for fc in range(8):
                    p1 = [ps.tile([128, 512], f32, tag="ps", name="ps")
                          for _ in range(4)]
                    for mc in range(2):
                        for j in range(4):
                            mm(p1[j][:, 0:256],
                               mw1T[:, mc * 1024 + fc * 128:
                                    mc * 1024 + fc * 128 + 128],
                               mhln[:, j * 512 + mc * 256:j * 512 + mc * 256 + 256],
                               mc == 0, mc == 1)
                    for j in range(4):
                        y1g = y1p.tile([128, 256], bf, tag="y1", name="y1")
                        nc.scalar.activation(y1g[:], p1[j][:, 0:256], AF.Gelu,
                                             bias=mb1f[:, fc:fc + 1])
                        for lc in range(2):
                            mm(py2[j][:, lc * 256:lc * 256 + 256],
                               y1g[:, lc * 128:lc * 128 + 128],
                               mw2T[:, fc * 256:(fc + 1) * 256],
                               fc == 0, False)
                # bias + residual -> X = exp(wn pre-softmax)
                for j in range(4):
                    h = h0 + j
                    for lc in range(2):
                        mm(py2[j][:, lc * 256:lc * 256 + 256], ones2[0:2, 0:128],
                           mrowb[0:2, 256:512], False, True)
                        wnpre = scrq.tile([128, 256], bf, tag="wnpre", name="wnpre")
                        nc.vector.tensor_add(
                            wnpre[:], py2[j][:, lc * 256:lc * 256 + 256],
                            hm[:, j * 512 + lc * 256:j * 512 + lc * 256 + 256])
                        nc.scalar.activation(
                            X_bf[:, h * 512 + lc * 256:h * 512 + lc * 256 + 256],
                            wnpre[:], AF.Exp)
                # -- column sums Z[m] of X (2 heads per PSUM bank) --
                for jp in range(2):
                    pz = psy.tile([128, 512], f32, tag="psy", name="psy")
                    for hh in range(2):
                        h = h0 + jp * 2 + hh
                        for lc in range(2):
                            mm(pz[0:1, hh * 256:hh * 256 + 256],
                               onescol[0:128, 0:1],
                               X_bf[:, h * 512 + lc * 256:h * 512 + lc * 256 + 256],
                               lc == 0, lc == 1)
                    h2 = h0 + jp * 2
                    nc.vector.tensor_copy(
                        zrowb_t[0:1, h2 * 256:h2 * 256 + 512], pz[0:1, 0:512])
            # Z-row -> Z-cols on PE, then ONE batched reciprocal
            pzT = psy.tile([128, 512], f32, tag="psy", name="psy")
            for h in range(H):
                for mc in range(2):
                    mm(pzT[0:128, h * 2 + mc:h * 2 + mc + 1],
                       zrowb_t[0:2, h * 256 + mc * 128:h * 256 + mc * 128 + 128],
                       e0[0:2, 0:1], True, True)
            zcols = zp.tile([128, 16], f32, tag="zcols", name="zcols")
            nc.vector.tensor_copy(zcols[:], pzT[0:128, 0:16])
            nc.vector.reciprocal(izc_t[:], zcols[:])

            # ---- o_new (TM) + residual -> hres ----
            hres = hpool.tile([128, 2 * C], f32, tag="h", name="h")
            for h in range(H):
                for mc in range(2):
                    p = ps.tile([128, 512], f32, tag="ps", name="ps")
                    for lc in range(2):
                        mm(p[:, 0:96],
                           X_bf[:, h * 512 + lc * 256 + mc * 128:
                                h * 512 + lc * 256 + mc * 128 + 128],
                           y_bf[:, lc * C + 96 * h:lc * C + 96 * h + 96],
                           lc == 0, False)
                    mm(p[:, 0:96],
                       zrowb_t[0:2, h * 256 + mc * 128:h * 256 + mc * 128 + 128],
                       rowb_t[0:2, 96 * h:96 * h + 96], False, True)
                    nc.vector.scalar_tensor_tensor(
                        hres[:, mc * C + 96 * h:mc * C + 96 * h + 96],
                        p[:, 0:96], izc_t[:, h * 2 + mc:h * 2 + mc + 1],
                        h_t[:, mc * C + 96 * h:mc * C + 96 * h + 96],
                        ALU.mult, ALU.add)

            if dbg_d is not None and li == DBG_LAYERS - 1:
                dcast = hpool.tile([128, 2 * C], f32, tag="h", name="dcast")
                nc.vector.tensor_copy(dcast[:], y_bf[:])
                nc.sync.dma_start(dbg_d[:, 0:1536], dcast[:])
                nc.sync.dma_start(dbg_d[:, 1536:3072], hres[:])
                dcast2 = hpool.tile([128, 2 * C], f32, tag="h", name="dcast2")
                nc.vector.tensor_copy(dcast2[:], E_bf[:, 0:1536])
                nc.sync.dma_start(dbg_d[:, 3072:4608], dcast2[:])
                dcast3 = hpool.tile([128, 2 * C], f32, tag="h", name="dcast3")
                nc.vector.tensor_copy(dcast3[:], X_bf[:, 0:1536])
                nc.sync.dma_start(dbg_d[:, 4608:6144], dcast3[:])

            # ---- outer LN2 + MLP ----
            hln2 = ln_transpose_outer(hres, 12, 18, eps5, "hln", hlnp)
            h_next = hpool.tile([128, 2 * C], f32, tag="h", name="h")
            # py2 banks: A=lc0[0:512], B=lc1[0:512], Cb=[lc0 512:768|lc1 512:768]
            pA = psy.tile([128, 512], f32, tag="psy", name="psy")
            pB = psy.tile([128, 512], f32, tag="psy", name="psy")
            pC1 = psy.tile([128, 512], f32, tag="psy", name="psy")
            pC2 = psy.tile([128, 512], f32, tag="psy", name="psy")
            y2tgt = [(pA, 0, 512, 0, 0), (pC1, 0, 256, 0, 512),
                     (pB, 0, 512, 1, 0), (pC2, 0, 256, 1, 512)]
            for piece in range(4):
                w1p = ww1.tile([128, 6 * C], bf, tag="ww1", name="ww1")
                nc.sync.dma_start(
                    w1p[:], dram["w1T"][li, :, piece * 4608:(piece + 1) * 4608])
                w2p = ww2.tile([128, 6 * C], bf, tag="ww2", name="ww2")
                nc.sync.dma_start(
                    w2p[:], dram["w2T"][li, :, piece * 4608:(piece + 1) * 4608])
                for fcl in range(6):
                    fc = piece * 6 + fcl
                    p1 = ps.tile([128, 512], f32, tag="ps", name="ps")
                    for cb in range(6):
                        mm(p1[:, 0:256],
                           w1p[:, fcl * C + cb * 128:fcl * C + cb * 128 + 128],
                           hln2[:, cb * 256:(cb + 1) * 256], cb == 0, cb == 5)
                    y1g = y1p.tile([128, 256], bf, tag="y1", name="y1")
                    nc.scalar.activation(y1g[:], p1[:, 0:256], AF.Gelu,
                                         bias=b1f_t[:, fc:fc + 1])
                    for pt, po, nw, lc, n0 in y2tgt:
                        mm(pt[:, po:po + nw], y1g[:, lc * 128:lc * 128 + 128],
                           w2p[:, fcl * C + n0:fcl * C + n0 + nw],
                           fc == 0, False)
            for pt, po, nw, lc, n0 in y2tgt:
                mm(pt[:, po:po + nw], ones2[0:2, 0:128],
                   rowb_t[0:2, C + n0:C + n0 + nw], False, True)
                nc.vector.tensor_add(
                    h_next[:, lc * C + n0:lc * C + n0 + nw], pt[:, po:po + nw],
                    hres[:, lc * C + n0:lc * C + n0 + nw])
            h_t = h_next

        if dbg_d is not None:
            nc.sync.dma_start(dbg_d[:, 6144:7680], h_t[:])
        if not DBG_EPI:
            logits = scr.tile([10, 1], f32, tag="logits", name="logits")
            nc.gpsimd.memset(logits[:], 0.0)
            nc.sync.dma_start(out_d[:], logits[:])
            return
        # ---------------- epilogue ----------------
        # pooled^T (FM fold [128, 6]) = mean over tokens
        pooled = scr.tile([128, 8], f32, tag="pooled", name="pooled")
        hbf = scr.tile([128, 2 * C], bf, tag="lnsub", name="hfin")
        nc.vector.tensor_copy(hbf[:], h_t[:])
        for cb in range(6):
            p = psy.tile([128, 512], f32, tag="psy", name="psy")
            for lc in range(2):
                mm(p[:, 0:1], hbf[:, lc * C + cb * 128:lc * C + cb * 128 + 128],
                   onescol[0:128, 0:1], lc == 0, lc == 1)
            nc.vector.tensor_scalar_mul(pooled[:, cb:cb + 1], p[:, 0:1],
                                        1.0 / L)
        # LN over all 768 (partition+fold): stats via f32 matmuls
        sq = scr.tile([128, 8], bf, tag="pooledsq", name="pooledsq")
        sqa = stp.tile([128, 1], f32, tag="st", name="st")
        nc.vector.scalar_tensor_tensor(
            sq[:, 0:6], pooled[:, 0:6], 1.0, pooled[:, 0:6], ALU.mult,
            ALU.mult, accum_out=sqa[:])
        sqab = stp.tile([128, 1], bf, tag="stb", name="stb")
        nc.vector.tensor_copy(sqab[:], sqa[:])
        sma = stp.tile([128, 1], f32, tag="st", name="st")
        nc.vector.reduce_sum(sma[:], pooled[:, 0:6], axis=X_AXIS)
        smab = stp.tile([128, 1], bf, tag="stb", name="stb")
        nc.vector.tensor_copy(smab[:], sma[:])
        pst = psy.tile([128, 512], f32, tag="psy", name="psy")
        mm(pst[0:1, 0:1], smab[:], onescol[0:128, 0:1], True, True)
        mm(pst[0:1, 1:2], sqab[:], onescol[0:128, 0:1], True, True)
        stat2 = zp.tile([1, 2], f32, tag="st2", name="st2")
        nc.vector.tensor_copy(stat2[:], pst[0:1, 0:2])
        mean = zp.tile([1, 2], f32, tag="mv2e", name="mv2e")
        nc.vector.tensor_scalar_mul(mean[:, 0:1], stat2[:, 0:1], 1.0 / C)
        m2 = zp.tile([1, 1], f32, tag="m2", name="m2")
        nc.vector.tensor_tensor(m2[:], mean[:, 0:1], mean[:, 0:1], ALU.mult)
        var = zp.tile([1, 1], f32, tag="var", name="var")
        nc.vector.scalar_tensor_tensor(var[:], stat2[:, 1:2], 1.0 / C, m2[:],
                                       ALU.mult, ALU.subtract)
        sde = zp.tile([1, 1], f32, tag="sde", name="sde")
        nc.scalar.activation(sde[:], var[:], AF.Sqrt, bias=eps5[0:1, :])
        rr = zp.tile([1, 1], f32, tag="rr", name="rr")
        nc.vector.reciprocal(rr[:], sde[:])
        mrb = zp.tile([2, 2], bf, tag="mrb", name="mrb")
        nc.gpsimd.memset(mrb[:], 0.0)
        nc.vector.tensor_copy(mrb[0:1, 0:1], mean[:, 0:1])
        nc.vector.tensor_copy(mrb[0:1, 1:2], rr[:])
        # broadcast mean, rstd to [128, 1] via K=2 bf16 matmul
        pbc = psy.tile([128, 512], f32, tag="psy", name="psy")
        mm(pbc[0:128, 0:2], ones2[0:2, 0:128], mrb[0:2, 0:2], True, True)
        mbc = stp.tile([128, 1], f32, tag="st", name="st")
        nc.vector.tensor_copy(mbc[:], pbc[0:128, 0:1])
        rbc = stp.tile([128, 1], f32, tag="st", name="st")
        nc.vector.tensor_copy(rbc[:], pbc[0:128, 1:2])
        pn = scr.tile([128, 8], f32, tag="pn", name="pn")
        nc.vector.tensor_scalar(pn[:, 0:6], pooled[:, 0:6], mbc[:], rbc[:],
                                ALU.subtract, ALU.mult)
        nc.vector.tensor_tensor(pn[:, 0:6], pn[:, 0:6], normgb[:, 0:6],
                                ALU.mult)
        nc.vector.tensor_add(pn[:, 0:6], pn[:, 0:6], normgb[:, 6:12])
        # head (f32 matmuls)
        ph = psy.tile([128, 512], f32, tag="psy", name="psy")
        for cb in range(6):
            mm(ph[0:10, 0:1], headwT[:, cb * 10:(cb + 1) * 10],
               pn[:, cb:cb + 1], cb == 0, cb == 5)
        logits = scr.tile([10, 1], f32, tag="logits", name="logits")
        nc.vector.tensor_add(logits[:], ph[0:10, 0:1], headb[0:10, 0:1])
        nc.sync.dma_start(out_d[:], logits[:])


_NC_CACHE = {}
TRACE = False
TRACE_TMPDIR = None
LAST = {}


def _get_nc(in_map):
    key = "k"
    if key not in _NC_CACHE:
        _NC_CACHE[key] = build(in_map)
    return _NC_CACHE[key]


def kernel(**inputs):
    per_core = marshal(inputs)
    nc = _get_nc(per_core[0])
    kw = {}
    if TRACE and TRACE_TMPDIR:
        kw["tmpdir"] = TRACE_TMPDIR
    res = run_bass_kernel_spmd(nc, per_core, core_ids=list(range(N_CORES)),
                               trace=TRACE, **kw)
    LAST["exec_time_ns"] = res.exec_time_ns
    out = np.stack([res.results[b]["out"][:, 0] for b in range(N_CORES)])
    return out.astype(np.float32)


# revision 40
# speedup vs baseline: 1.0261x; 1.0261x over previous
"""Trainium2 Bass kernel for nn_CIFARViT: 8-layer ViT with a per-head
mini-transformer over attention maps. Data-parallel: one batch element
per NeuronCore (8 cores), full inputs in / full outputs out.

v2 (engine-balance rewrite):
  - softmax max-subtraction dropped (logits verified |s| < 2).
  - all per-head [128,1] stat ops batched into multi-column tiles; one
    reciprocal per stage instead of per (head, chunk).
  - rsqrt = Exp(-0.5*Ln(v+eps)) on ACT (natural_log_exp table set, same
    set as the score exps -> no table thrash).
  - LayerNorm stats via bn_stats/bn_aggr.
  - gamma/beta + PSUM->SBUF moves fused into ACT Identity ops; plain
    copies alternate DVE/ACT to balance engine load.
  - mini-transformer restructured into 4-head stage-batched groups.
  - per-head K=96 proj (kills the 24-way oT copy split).
  - wn column-sum reciprocals: transpose Z rows to columns on PE first,
    then ONE [128,16] reciprocal (was 64 serial [1,256] recips).
"""
import sys

sys.path.insert(0, "/opt/trn_rl_repo")

import numpy as np
import ml_dtypes

import concourse.bass as bass
import concourse.mybir as mybir
import concourse.tile as tile
from concourse import bacc
from concourse.bass_utils import run_bass_kernel_spmd

BF = ml_dtypes.bfloat16
F32 = np.float32
AF = mybir.ActivationFunctionType
ALU = mybir.AluOpType
bf = mybir.dt.bfloat16
f32 = mybir.dt.float32

H = 8
HD = 96
C = 768
L = 256
D_LAYERS = 8
F = 3072

N_CORES = 8
X_AXIS = mybir.AxisListType.X
import os
DBG_LAYERS = int(os.environ.get("KLAYERS", "8"))
DBG_TAP = os.environ.get("KTAP", "") == "1"
DBG_EPI = os.environ.get("KEPI", "1") == "1"


def _fold(wt):
    """[R, Cc] with R = 128*T -> [128, T*Cc] partition fold."""
    R, Cc = wt.shape
    T = R // 128
    return np.ascontiguousarray(
        wt.reshape(T, 128, Cc).transpose(1, 0, 2).reshape(128, T * Cc)
    )


def _foldv(v):
    T = v.shape[0] // 128
    return np.ascontiguousarray(v.reshape(T, 128).T)


def marshal(inputs):
    inp = {k: np.asarray(v) for k, v in inputs.items()}
    sh = {}
    qk_l, v_l, proj_l, w1_l, w2_l, lnp_l, b1_l, rowb_l = ([] for _ in range(8))
    for i in range(D_LAYERS):
        qkvT = inp["qkv_w"][i].T.astype(BF)  # [768, 2304]
        qkf = _fold(qkvT)  # [128, 6*2304]
        img = np.zeros((128, 4 * 2304), dtype=BF)
        for hp in range(4):
            for cb in range(6):
                for hh in range(2):
                    h = hp * 2 + hh
                    base = hp * 2304 + cb * 384 + hh * 192
                    img[:, base:base + 96] = \
                        qkf[:, cb * 2304 + 96 * h: cb * 2304 + 96 * h + 96]
                    img[:, base + 96:base + 192] = \
                        qkf[:, cb * 2304 + 768 + 96 * h: cb * 2304 + 768 + 96 * h + 96]
        qk_l.append(img)
        v_l.append(_fold(np.ascontiguousarray(qkvT[:, 1536:2304])))
        # per-head proj rows: [96, 8*768]  (head h rows 96h..96h+96 of W^T)
        pw = inp["proj_w"][i].T.astype(BF)  # [768, 768]
        ph = np.zeros((96, 8 * 768), dtype=BF)
        for h in range(8):
            ph[:, h * 768:(h + 1) * 768] = pw[96 * h:96 * h + 96, :]
        proj_l.append(ph)
        w1T = inp["mlp_w1"][i].T.astype(BF)  # [768, 3072]
        w1_l.append(np.ascontiguousarray(
            w1T.reshape(6, 128, 24, 128).transpose(1, 2, 0, 3).reshape(128, 24 * 768)))
        w2_l.append(_fold(inp["mlp_w2"][i].T.astype(BF)))  # [128, 24*768]
        lnp_l.append(np.concatenate(
            [_foldv(inp[k][i].astype(F32))
             for k in ("ln1_g", "ln1_b", "ln2_g", "ln2_b")], axis=1))
        b1_l.append(_foldv(inp["mlp_b1"][i].astype(F32)))
        rb = np.zeros((2, 2 * C), dtype=BF)
        rb[0, :C] = inp["proj_b"][i].astype(BF)
        rb[0, C:] = inp["mlp_b2"][i].astype(BF)
        rowb_l.append(rb)
    sh["qk_img"] = np.stack(qk_l)
    sh["v_img"] = np.stack(v_l)
    sh["projT"] = np.stack(proj_l)
    sh["w1T"] = np.stack(w1_l)
    sh["w2T"] = np.stack(w2_l)
    sh["lnp"] = np.stack(lnp_l)
    sh["b1f"] = np.stack(b1_l)
    sh["rowb"] = np.stack(rowb_l)

    sh["mqkvT"] = _fold(inp["m_qkv_w"].T.astype(BF))   # [128, 2*768]
    sh["mprojT"] = _fold(inp["m_proj_w"].T.astype(BF))  # [128, 2*256]
    sh["mw1T"] = _fold(inp["m_mlp_w1"].T.astype(BF))   # [128, 2*1024]
    sh["mw2T"] = _fold(inp["m_mlp_w2"].T.astype(BF))   # [128, 8*256]
    sh["mlnp"] = np.concatenate(
        [_foldv(inp[k].astype(F32))
         for k in ("m_ln1_g", "m_ln1_b", "m_ln2_g", "m_ln2_b")], axis=1)
    sh["mb1f"] = _foldv(inp["m_mlp_b1"].astype(F32))   # [128, 8]
    mrb_ = np.zeros((2, 512), dtype=BF)
    mrb_[0, :256] = inp["m_proj_b"].astype(BF)
    mrb_[0, 256:] = inp["m_mlp_b2"].astype(BF)
    sh["mrowb"] = mrb_

    sh["pwT"] = np.ascontiguousarray(inp["patch_w"].reshape(C, 12).T.astype(BF))
    pos = inp["pos_emb"][0].astype(F32) + inp["patch_b"][None, :].astype(F32)
    sh["pos"] = _fold(pos)  # [128, 2*768]
    sh["normgb"] = np.concatenate(
        [_foldv(inp["norm_g"].astype(F32)), _foldv(inp["norm_b"].astype(F32))],
        axis=1)  # [128, 12]
    sh["headwT"] = _fold(inp["head_w"].T.astype(F32))  # [128, 6*10]
    sh["headb"] = inp["head_b"].astype(F32).reshape(10, 1)
    sh["ident"] = np.eye(128, dtype=BF)
    sh["onescol"] = np.ones((128, 1), dtype=BF)
    sh["ones2"] = np.ones((2, 128), dtype=BF)
    e0_ = np.zeros((2, 1), dtype=BF); e0_[0, 0] = 1.0
    sh["e0"] = e0_

    x = inp["x"].astype(F32)
    per_core = []
    for b in range(N_CORES):
        pt = (x[b].reshape(3, 16, 2, 16, 2).transpose(0, 2, 4, 1, 3)
              .reshape(12, 256).astype(BF))
        m = dict(sh)
        m["patchesT"] = np.ascontiguousarray(pt)
        per_core.append(m)
    return per_core


DT_MAP = {np.dtype(BF): bf, np.dtype(np.float32): f32}


def build(in_map):
    nc = bacc.Bacc("TRN2", target_bir_lowering=False, debug=False,
                   num_devices=N_CORES)
    dram = {k: nc.dram_tensor(k, v.shape, DT_MAP[v.dtype], kind="ExternalInput")
            for k, v in in_map.items()}
    out_d = nc.dram_tensor("out", (10, 1), f32, kind="ExternalOutput")
    dbg_d = nc.dram_tensor("dbg", (128, 10 * C), f32,
                           kind="ExternalOutput") if DBG_TAP else None
    with tile.TileContext(nc) as tc:
        _body(nc, tc, dram, out_d, dbg_d)
    nc.compile()
    return nc


def _body(nc, tc, dram, out_d, dbg_d=None):
    import contextlib
    ctx = contextlib.ExitStack()
    with ctx:
        P = lambda name, bufs=1, space="SBUF": ctx.enter_context(
            tc.tile_pool(name=name, bufs=bufs, space=space))
        cpool = P("const")

        def cload(name):
            arr = dram[name]
            t = cpool.tile(list(arr.shape), arr.dtype, tag=name, name=name)
            nc.sync.dma_start(t[:], arr[:])
            return t

        mqkvT = cload("mqkvT")
        mprojT = cload("mprojT")
        mw1T = cload("mw1T")
        mw2T = cload("mw2T")
        mlnp = cload("mlnp")
        mb1f = cload("mb1f")
        mrowb = cload("mrowb")
        ident = cload("ident")
        onescol = cload("onescol")
        ones2 = cload("ones2")
        e0 = cload("e0")
        normgb = cload("normgb")
        headwT = cload("headwT")
        headb = cload("headb")
        pwT = cload("pwT")
        patchesT = cload("patchesT")

        NDIAG = 3
        dtiles = []
        for j in range(NDIAG):
            t = cpool.tile([128, 512], bf, tag=f"diag{j}", name=f"diag{j}")
            nc.gpsimd.memset(t[:], 0.0)
            dtiles.append(t)
        dctr = [0]

        eps6 = cpool.tile([128, 1], f32, tag="eps6", name="eps6")
        nc.gpsimd.memset(eps6[:], 1e-6)
        eps5 = cpool.tile([128, 1], f32, tag="eps5", name="eps5")
        nc.gpsimd.memset(eps5[:], 1e-5)

        def build_diag(s0, s1):
            d = dtiles[dctr[0] % NDIAG]
            dctr[0] += 1
            nc.vector.tensor_scalar_mul(d[:, 0:128], ident[:], s0)
            nc.vector.tensor_scalar_mul(d[:, 384:512], ident[:], s1)
            return d

        # alternate PSUM->SBUF moves between DVE and ACT
        cctr = [0]

        def pcopy(dst, src):
            cctr[0] += 1
            if cctr[0] % 3 != 0:
                nc.vector.tensor_copy(dst, src)
            else:
                nc.scalar.activation(dst, src, AF.Copy)

        hpool = P("h", bufs=3)
        ps = P("ps", bufs=4, space="PSUM")
        psy = P("psy", bufs=4, space="PSUM")
        stp = P("st", bufs=8)
        bsp = P("bst", bufs=16)
        wqk = P("wqk", bufs=2)
        wv = P("wv", bufs=1)
        wproj = P("wproj", bufs=1)
        ww1 = P("ww1", bufs=2)
        ww2 = P("ww2", bufs=2)
        hlnp = P("hln", bufs=1)
        qkp = P("qkt", bufs=1)
        vtp = P("vt", bufs=1)
        ebp = P("eb", bufs=1)
        atp = P("at", bufs=1)
        ybp = P("yb", bufs=1)
        xbp = P("xb", bufs=1)
        scr = P("scr", bufs=2)
        scrq = P("scrq", bufs=3)
        sqp = P("sqp", bufs=2)
        gp = P("gp", bufs=1)
        y1p = P("y1", bufs=3)
        zp = P("zp", bufs=2)
        zrp = P("zrp", bufs=1)
        rbp = P("rbp", bufs=1)

        def bst(w=8):
            return bsp.tile([128, w], f32, tag=f"bst{w}", name="bst")

        def mm(out, lhsT, rhs, start, stop):
            nc.tensor.matmul(out, lhsT, rhs, start=start, stop=stop)

        # ---------------- prologue: patch embed ----------------
        h_t = hpool.tile([128, 2 * C], f32, tag="h", name="h")
        pos_t = cpool.tile([128, 2 * C], f32, tag="pos", name="pos")
        nc.sync.dma_start(pos_t[:], dram["pos"][:])
        for lc in range(2):
            for n0, nw in ((0, 512), (512, 256)):
                p = ps.tile([128, 512], f32, tag="ps", name="ps")
                mm(p[:, 0:nw], patchesT[0:12, lc * 128:lc * 128 + 128],
                   pwT[0:12, n0:n0 + nw], True, True)
                nc.vector.tensor_add(
                    h_t[:, lc * C + n0:lc * C + n0 + nw], p[:, 0:nw],
                    pos_t[:, lc * C + n0:lc * C + n0 + nw])

        # ---------------- layers ----------------
        for li in range(DBG_LAYERS):
            lnp_t = scr.tile([128, 24], f32, tag="lnp", name="lnp")
            nc.sync.dma_start(lnp_t[:], dram["lnp"][li])
            b1f_t = scr.tile([128, 24], f32, tag="b1f", name="b1f")
            nc.sync.dma_start(b1f_t[:], dram["b1f"][li])
            rowb_t = rbp.tile([2, 2 * C], bf, tag="rowb", name="rowb")
            nc.sync.dma_start(rowb_t[:], dram["rowb"][li])
            vt_w = wv.tile([128, 6 * C], bf, tag="wv", name="wv")
            nc.sync.dma_start(vt_w[:], dram["v_img"][li])
            projT = wproj.tile([96, 8 * C], bf, tag="wproj", name="wproj")
            nc.sync.dma_start(projT[:], dram["projT"][li])

            def ln_transpose_outer(src, g_col, b_col, epsv, tag, pool):
                """TM f32 [128, 2*768] -> LN'd FM bf16 [128, 6*256].
                bn_stats stats + rsqrt via gpsimd pow; gamma/beta fused
                into the PSUM->SBUF move on ACT."""
                dst = pool.tile([128, 6 * 256], bf, tag=tag)
                subs = scr.tile([128, 2 * C], bf, tag="lnsub", name="lnsub")
                sdv = bst(2)
                rs = bst(2)
                for lc in range(2):
                    sl = src[:, lc * C:(lc + 1) * C]
                    st3 = scrq.tile([128, 3, 6], f32, tag="st3", name="st3")
                    for sg in range(3):
                        nc.vector.bn_stats(
                            st3[:, sg, :], sl[:, sg * 256:(sg + 1) * 256])
                    mv2 = bsp.tile([128, 2], f32, tag="mv2", name="mv2")
                    nc.vector.bn_aggr(mv2[:, :], st3[:, :, :])
                    nc.scalar.activation(sdv[:, lc:lc + 1], mv2[:, 1:2],
                                         AF.Sqrt, bias=epsv[:])
                    nc.vector.tensor_scalar_sub(
                        subs[:, lc * C:(lc + 1) * C], sl, mv2[:, 0:1])
                nc.vector.reciprocal(rs[:], sdv[:])
                d = build_diag(rs[:, 0:1], rs[:, 1:2])
                for cb in range(6):
                    p = ps.tile([128, 512], f32, tag="ps", name="ps")
                    for lc in range(2):
                        mm(p[:, lc * 128:lc * 128 + 128],
                           subs[:, lc * C + cb * 128:lc * C + cb * 128 + 128],
                           d[:, lc * 384:lc * 384 + 128], True, True)
                    nc.scalar.activation(
                        dst[:, cb * 256:(cb + 1) * 256], p[:, 0:256],
                        AF.Identity,
                        bias=lnp_t[:, b_col + cb:b_col + cb + 1],
                        scale=lnp_t[:, g_col + cb:g_col + cb + 1])
                return dst

            hln = ln_transpose_outer(h_t, 0, 6, eps6, "hln", hlnp)

            # ---- qkv: q^T|k^T per head [96, 512]; v token-major ----
            qk_bf = qkp.tile([128, 4096], bf, tag="qk", name="qk")
            for hp in range(4):
                qkw = wqk.tile([128, 2304], bf, tag="wqk", name="wqk")
                nc.sync.dma_start(
                    qkw[:], dram["qk_img"][li, :, hp * 2304:(hp + 1) * 2304])
                for hh in range(2):
                    h = hp * 2 + hh
                    p = ps.tile([128, 512], f32, tag="ps", name="ps")
                    for half in range(2):
                        for cb in range(6):
                            lh = qkw[:, cb * 384 + hh * 192 + half * 96:
                                     cb * 384 + hh * 192 + half * 96 + 96]
                            mm(p[0:96, half * 256:half * 256 + 256], lh,
                               hln[:, cb * 256:(cb + 1) * 256],
                               cb == 0, cb == 5)
                    pcopy(qk_bf[0:96, h * 512:(h + 1) * 512], p[0:96, :])

            # ---- attention scores -> E (exp, no max-sub) ----
            E_bf = ebp.tile([128, H * 512], bf, tag="eb", name="eb")
            izz = zp.tile([128, 32], f32, tag="izz", name="izz")
            for h in range(H):
                p = ps.tile([128, 512], f32, tag="ps", name="ps")
                for lc in range(2):
                    mm(p[:, lc * 256:lc * 256 + 256],
                       qk_bf[0:96, h * 512 + lc * 128:h * 512 + lc * 128 + 128],
                       qk_bf[0:96, h * 512 + 256:h * 512 + 512], True, True)
                for lc in range(2):
                    nc.scalar.activation(
                        E_bf[:, h * 512 + lc * 256:h * 512 + lc * 256 + 256],
                        p[:, lc * 256:lc * 256 + 256], AF.Exp,
                        scale=HD ** -0.5,
                        accum_out=izz[:, h * 2 + lc:h * 2 + lc + 1])
            nc.vector.reciprocal(izz[:, 16:32], izz[:, 0:16])

            # ---- v token-major (PE busy while recip runs) ----
            vt_bf = vtp.tile([128, 2 * C], bf, tag="vt", name="vt")
            for lc in range(2):
                for n0, nw in ((0, 512), (512, 256)):
                    p = ps.tile([128, 512], f32, tag="ps", name="ps")
                    for cb in range(6):
                        mm(p[:, 0:nw],
                           hln[:, cb * 256 + lc * 128:cb * 256 + lc * 128 + 128],
                           vt_w[:, cb * C + n0:cb * C + n0 + nw],
                           cb == 0, cb == 5)
                    pcopy(vt_bf[:, lc * C + n0:lc * C + n0 + nw], p[:, 0:nw])

            # ---- aT = (E/Z)^T via transpose-with-diag ----
            aT_bf = atp.tile([128, H * 512], bf, tag="at", name="at")
            for h in range(H):
                d = build_diag(izz[:, 16 + 2 * h:17 + 2 * h],
                               izz[:, 17 + 2 * h:18 + 2 * h])
                pa = ps.tile([128, 512], f32, tag="ps", name="ps")
                for mc in range(2):
                    for lc in range(2):
                        mm(pa[:, mc * 256 + lc * 128:mc * 256 + lc * 128 + 128],
                           E_bf[:, h * 512 + lc * 256 + mc * 128:
                                h * 512 + lc * 256 + mc * 128 + 128],
                           d[:, lc * 384:lc * 384 + 128], True, True)
                pcopy(aT_bf[:, h * 512:(h + 1) * 512], pa[:])

            # ---- attnV -> oT ([96, 8*256], head h at cols h*256) ----
            oT_bf = qkp.tile([128, 4096], bf, tag="qk", name="ot")
            for hq in range(4):
                po = ps.tile([128, 512], f32, tag="ps", name="ps")
                for hh in range(2):
                    h = hq * 2 + hh
                    for mc in range(2):
                        mm(po[0:96, hh * 256:hh * 256 + 256],
                           vt_bf[:, mc * C + 96 * h:mc * C + 96 * h + 96],
                           aT_bf[:, h * 512 + mc * 256:h * 512 + mc * 256 + 256],
                           mc == 0, mc == 1)
                pcopy(oT_bf[0:96, hq * 512:(hq + 1) * 512], po[0:96, :])

            # ---- proj -> y (TM bf16), per-head K=96 accumulation ----
            y_bf = ybp.tile([128, 2 * C], bf, tag="yb", name="yb")
            for lc in range(2):
                for n0, nw in ((0, 512), (512, 256)):
                    p = ps.tile([128, 512], f32, tag="ps", name="ps")
                    for h in range(H):
                        mm(p[:, 0:nw],
                           oT_bf[0:96, h * 256 + lc * 128:h * 256 + lc * 128 + 128],
                           projT[0:96, h * C + n0:h * C + n0 + nw],
                           h == 0, h == 7)
                    pcopy(y_bf[:, lc * C + n0:lc * C + n0 + nw], p[:, 0:nw])

            # ---- mini transformer: 2 groups of 4 heads, stage-batched ----
            X_bf = xbp.tile([128, H * 512], bf, tag="xb", name="xb")
            zrowb_t = zrp.tile([2, H * 256], bf, tag="zrowb", name="zrowb")
            nc.gpsimd.memset(zrowb_t[:], 0.0)
            mizz = zp.tile([128, 32], f32, tag="mizz", name="mizz")
            izc_t = zp.tile([128, 16], f32, tag="izc", name="izc")
            for g in range(2):
                h0 = 4 * g
                izg = izz[:, 16 + 8 * g:24 + 8 * g]   # 1/Z outer, 8 cols
                zg = izz[:, 8 * g:8 + 8 * g]          # Z outer
                # -- mini LN1 on a = E/Z: batched stats --
                sqE = bsp.tile([128, 8], f32, tag="sqE", name="sqE")
                for i in range(8):
                    h, lc = h0 + i // 2, i % 2
                    sq = scrq.tile([128, 256], bf, tag="sq", name="sq")
                    esl = E_bf[:, h * 512 + lc * 256:h * 512 + lc * 256 + 256]
                    nc.vector.scalar_tensor_tensor(
                        sq[:], esl, 1.0, esl, ALU.mult, ALU.mult,
                        accum_out=sqE[:, i:i + 1])
                iz2 = bst()
                nc.vector.tensor_tensor(iz2[:], izg, izg, ALU.mult)
                t1 = bst()
                nc.vector.tensor_tensor(t1[:], sqE[:], iz2[:], ALU.mult)
                var = bst()
                nc.vector.tensor_scalar(var[:], t1[:], 1.0 / L, 1.0 / (L * L),
                                        ALU.mult, ALU.subtract)
                sd8 = bst()
                nc.scalar.activation(sd8[:], var[:], AF.Sqrt, bias=eps6[:])
                ra = bst()
                nc.vector.reciprocal(ra[:], sd8[:])
                rz = bst()
                nc.vector.tensor_tensor(rz[:], ra[:], izg, ALU.mult)
                zc = bst()
                nc.vector.tensor_scalar_mul(zc[:], zg, 1.0 / L)
                esub = gp.tile([128, 2048], bf, tag="esub", name="esub")
                for i in range(8):
                    h, lc = h0 + i // 2, i % 2
                    nc.vector.tensor_scalar_sub(
                        esub[:, i * 256:(i + 1) * 256],
                        E_bf[:, h * 512 + lc * 256:h * 512 + lc * 256 + 256],
                        zc[:, i:i + 1])
                wnlnT = gp.tile([128, 2048], bf, tag="wnln", name="wnln")
                for j in range(4):
                    d = build_diag(rz[:, 2 * j:2 * j + 1], rz[:, 2 * j + 1:2 * j + 2])
                    pw_ = ps.tile([128, 512], f32, tag="ps", name="ps")
                    for mc in range(2):
                        for lc in range(2):
                            mm(pw_[:, mc * 256 + lc * 128:mc * 256 + lc * 128 + 128],
                               esub[:, j * 512 + lc * 256 + mc * 128:
                                    j * 512 + lc * 256 + mc * 128 + 128],
                               d[:, lc * 384:lc * 384 + 128], True, True)
                    for mc in range(2):
                        nc.scalar.activation(
                            wnlnT[:, j * 512 + mc * 256:j * 512 + mc * 256 + 256],
                            pw_[:, mc * 256:mc * 256 + 256], AF.Identity,
                            bias=mlnp[:, 2 + mc:3 + mc],
                            scale=mlnp[:, 0 + mc:1 + mc])
                # -- mini qkv (q^T|k^T FM per head) --
                mqk = gp.tile([128, 4096], bf, tag="mqk", name="mqk")
                for dc in range(2):
                    pq = [ps.tile([128, 512], f32, tag="ps", name="ps")
                          for _ in range(4)]
                    for half in range(2):
                        for mc in range(2):
                            for j in range(4):
                                mm(pq[j][:, half * 256:half * 256 + 256],
                                   mqkvT[:, mc * 768 + half * 256 + dc * 128:
                                         mc * 768 + half * 256 + dc * 128 + 128],
                                   wnlnT[:, j * 512 + mc * 256:
                                         j * 512 + mc * 256 + 256],
                                   mc == 0, mc == 1)
                    for j in range(4):
                        pcopy(mqk[:, j * 1024 + dc * 512:j * 1024 + dc * 512 + 512],
                              pq[j][:])
                # -- mini v (token-major) --
                mv_bf = gp.tile([128, 2048], bf, tag="mv", name="mv")
                for j in range(4):
                    pv = ps.tile([128, 512], f32, tag="ps", name="ps")
                    for lc in range(2):
                        for mc in range(2):
                            mm(pv[:, lc * 256:lc * 256 + 256],
                               wnlnT[:, j * 512 + mc * 256 + lc * 128:
                                     j * 512 + mc * 256 + lc * 128 + 128],
                               mqkvT[:, mc * 768 + 512:mc * 768 + 768],
                               mc == 0, mc == 1)
                    pcopy(mv_bf[:, j * 512:(j + 1) * 512], pv[:])
                # -- mini scores -> Em = exp (no max-sub) --
                Em = gp.tile([128, 2048], bf, tag="em", name="em")
                for j in range(4):
                    pe = ps.tile([128, 512], f32, tag="ps", name="ps")
                    for lc in range(2):
                        for dc in range(2):
                            mm(pe[:, lc * 256:lc * 256 + 256],
                               mqk[:, j * 1024 + dc * 512 + lc * 128:
                                   j * 1024 + dc * 512 + lc * 128 + 128],
                               mqk[:, j * 1024 + dc * 512 + 256:
                                   j * 1024 + dc * 512 + 512],
                               dc == 0, dc == 1)
                    for lc in range(2):
                        zcol = 8 * g + 2 * j + lc
                        nc.scalar.activation(
                            Em[:, j * 512 + lc * 256:j * 512 + lc * 256 + 256],
                            pe[:, lc * 256:lc * 256 + 256], AF.Exp,
                            scale=0.0625,
                            accum_out=mizz[:, zcol:zcol + 1])
                nc.vector.reciprocal(mizz[:, 16 + 8 * g:24 + 8 * g],
                                     mizz[:, 8 * g:8 + 8 * g])
                # -- amT = (Em/Z)^T --
                amT = gp.tile([128, 2048], bf, tag="mqk", name="amt")
                for j in range(4):
                    zcol = 16 + 8 * g + 2 * j
                    d = build_diag(mizz[:, zcol:zcol + 1],
                                   mizz[:, zcol + 1:zcol + 2])
                    pa = ps.tile([128, 512], f32, tag="ps", name="ps")
                    for mc in range(2):
                        for lc in range(2):
                            mm(pa[:, mc * 256 + lc * 128:mc * 256 + lc * 128 + 128],
                               Em[:, j * 512 + lc * 256 + mc * 128:
                                  j * 512 + lc * 256 + mc * 128 + 128],
                               d[:, lc * 384:lc * 384 + 128], True, True)
                    pcopy(amT[:, j * 512:(j + 1) * 512], pa[:])
                # -- mini attnV -> omT (FM) --
                omT = gp.tile([128, 2048], bf, tag="em", name="omt")
                for j in range(4):
                    po = ps.tile([128, 512], f32, tag="ps", name="ps")
                    for dc in range(2):
                        for mc in range(2):
                            mm(po[:, dc * 256:dc * 256 + 256],
                               mv_bf[:, j * 512 + mc * 256 + dc * 128:
                                     j * 512 + mc * 256 + dc * 128 + 128],
                               amT[:, j * 512 + mc * 256:j * 512 + mc * 256 + 256],
                               mc == 0, mc == 1)
                    pcopy(omT[:, j * 512:(j + 1) * 512], po[:])
                # -- hmini (TM f32) = omT.T@mprojT + aT.T@mprojT + pb --
                hm = gp.tile([128, 2048], f32, tag="hm", name="hm")
                for j in range(4):
                    h = h0 + j
                    for lc in range(2):
                        p = ps.tile([128, 512], f32, tag="ps", name="ps")
                        for mc in range(2):
                            mm(p[:, 0:256],
                               omT[:, j * 512 + mc * 256 + lc * 128:
                                   j * 512 + mc * 256 + lc * 128 + 128],
                               mprojT[:, mc * 256:(mc + 1) * 256],
                               mc == 0, False)
                        for mc in range(2):
                            mm(p[:, 0:256],
                               aT_bf[:, h * 512 + mc * 256 + lc * 128:
                                     h * 512 + mc * 256 + lc * 128 + 128],
                               mprojT[:, mc * 256:(mc + 1) * 256],
                               False, False)
                        mm(p[:, 0:256], ones2[0:2, 0:128],
                           mrowb[0:2, 0:256], False, True)
                        pcopy(hm[:, j * 512 + lc * 256:j * 512 + lc * 256 + 256],
                              p[:, 0:256])
                # -- mini LN2: bn stats, batched rsqrt --
                mvs2 = bsp.tile([128, 16], f32, tag="mvs2", name="mvs2")
                sd2 = bst()
                for i in range(8):
                    st6 = scrq.tile([128, 6], f32, tag="st6", name="st6")
                    nc.vector.bn_stats(
                        st6[:, :], hm[:, i * 256:(i + 1) * 256])
                    nc.vector.bn_aggr(mvs2[:, 2 * i:2 * i + 2], st6[:, :])
                    nc.scalar.activation(sd2[:, i:i + 1],
                                         mvs2[:, 2 * i + 1:2 * i + 2],
                                         AF.Sqrt, bias=eps5[:])
                r2 = bst()
                nc.vector.reciprocal(r2[:], sd2[:])
                hsub = gp.tile([128, 2048], bf, tag="esub", name="hsub")
                for i in range(8):
                    nc.vector.tensor_scalar_sub(
                        hsub[:, i * 256:(i + 1) * 256],
                        hm[:, i * 256:(i + 1) * 256], mvs2[:, 2 * i:2 * i + 1])
                mhln = gp.tile([128, 2048], bf, tag="wnln", name="mhln")
                for j in range(4):
                    d = build_diag(r2[:, 2 * j:2 * j + 1], r2[:, 2 * j + 1:2 * j + 2])
                    ph_ = ps.tile([128, 512], f32, tag="ps", name="ps")
                    for mc in range(2):
                        for lc in range(2):
                            mm(ph_[:, mc * 256 + lc * 128:mc * 256 + lc * 128 + 128],
                               hsub[:, j * 512 + lc * 256 + mc * 128:
                                    j * 512 + lc * 256 + mc * 128 + 128],
                               d[:, lc * 384:lc * 384 + 128], True, True)
                    for mc in range(2):
                        nc.scalar.activation(
                            mhln[:, j * 512 + mc * 256:j * 512 + mc * 256 + 256],
                            ph_[:, mc * 256:mc * 256 + 256], AF.Identity,
                            bias=mlnp[:, 6 + mc:7 + mc],
                            scale=mlnp[:, 4 + mc:5 + mc])
                # -- mini MLP (4 heads batched per fc; shared LDW) --
                py2 = [psy.tile([128, 512], f32, tag="psy", name="psy")
                       for _ in range(4)]
                for fc in range(8):
                    p1 = [ps.tile([128, 512], f32, tag="ps", name="ps")
                          for _ in range(4)]
                    for mc in range(2):
                        for j in range(4):
                            mm(p1[j][:, 0:256],
                               mw1T[:, mc * 1024 + fc * 128:
                                    mc * 1024 + fc * 128 + 128],
                               mhln[:, j * 512 + mc * 256:j * 512 + mc * 256 + 256],
                               mc == 0, mc == 1)
                    for j in range(4):
                        y1g = y1p.tile([128, 256], bf, tag="y1", name="y1")
                        nc.scalar.activation(y1g[:], p1[j][:, 0:256], AF.Gelu,
                                             bias=mb1f[:, fc:fc + 1])
                        for lc in range(2):
                            mm(py2[j][:, lc * 256:lc * 256 + 256],
                               y1g[:, lc * 128:lc * 128 + 128],
                               mw2T[:, fc * 256:(fc + 1) * 256],
                               fc == 0, False)
                # bias + residual -> X = exp(wn pre-softmax)
                for j in range(4):
                    h = h0 + j
                    for lc in range(2):
                        mm(py2[j][:, lc * 256:lc * 256 + 256], ones2[0:2, 0:128],
                           mrowb[0:2, 256:512], False, True)
                        wnpre = scrq.tile([128, 256], bf, tag="wnpre", name="wnpre")
                        nc.vector.tensor_add(
                            wnpre[:], py2[j][:, lc * 256:lc * 256 + 256],
                            hm[:, j * 512 + lc * 256:j * 512 + lc * 256 + 256])
                        nc.scalar.activation(
                            X_bf[:, h * 512 + lc * 256:h * 512 + lc * 256 + 256],
                            wnpre[:], AF.Exp)
                # -- column sums Z[m] of X (2 heads per PSUM bank) --
                for jp in range(2):
                    pz = psy.tile([128, 512], f32, tag="psy", name="psy")
                    for hh in range(2):
                        h = h0 + jp * 2 + hh
                        for lc in range(2):
                            mm(pz[0:1, hh * 256:hh * 256 + 256],
                               onescol[0:128, 0:1],
                               X_bf[:, h * 512 + lc * 256:h * 512 + lc * 256 + 256],
                               lc == 0, lc == 1)
                    h2 = h0 + jp * 2
                    nc.vector.tensor_copy(
                        zrowb_t[0:1, h2 * 256:h2 * 256 + 512], pz[0:1, 0:512])
            # Z-row -> Z-cols on PE, then ONE batched reciprocal
            pzT = psy.tile([128, 512], f32, tag="psy", name="psy")
            for h in range(H):
                for mc in range(2):
                    mm(pzT[0:128, h * 2 + mc:h * 2 + mc + 1],
                       zrowb_t[0:2, h * 256 + mc * 128:h * 256 + mc * 128 + 128],
                       e0[0:2, 0:1], True, True)
            zcols = zp.tile([128, 16], f32, tag="zcols", name="zcols")
            nc.vector.tensor_copy(zcols[:], pzT[0:128, 0:16])
            nc.vector.reciprocal(izc_t[:], zcols[:])

            # ---- o_new (TM) + residual -> hres ----
            hres = hpool.tile([128, 2 * C], f32, tag="h", name="h")
            for h in range(H):
                for mc in range(2):
                    p = ps.tile([128, 512], f32, tag="ps", name="ps")
                    for lc in range(2):
                        mm(p[:, 0:96],
                           X_bf[:, h * 512 + lc * 256 + mc * 128:
                                h * 512 + lc * 256 + mc * 128 + 128],
                           y_bf[:, lc * C + 96 * h:lc * C + 96 * h + 96],
                           lc == 0, False)
                    mm(p[:, 0:96],
                       zrowb_t[0:2, h * 256 + mc * 128:h * 256 + mc * 128 + 128],
                       rowb_t[0:2, 96 * h:96 * h + 96], False, True)
                    nc.vector.scalar_tensor_tensor(
                        hres[:, mc * C + 96 * h:mc * C + 96 * h + 96],
                        p[:, 0:96], izc_t[:, h * 2 + mc:h * 2 + mc + 1],
                        h_t[:, mc * C + 96 * h:mc * C + 96 * h + 96],
                        ALU.mult, ALU.add)

            if dbg_d is not None and li == DBG_LAYERS - 1:
                dcast = hpool.tile([128, 2 * C], f32, tag="h", name="dcast")
                nc.vector.tensor_copy(dcast[:], y_bf[:])
                nc.sync.dma_start(dbg_d[:, 0:1536], dcast[:])
                nc.sync.dma_start(dbg_d[:, 1536:3072], hres[:])
                dcast2 = hpool.tile([128, 2 * C], f32, tag="h", name="dcast2")
                nc.vector.tensor_copy(dcast2[:], E_bf[:, 0:1536])
                nc.sync.dma_start(dbg_d[:, 3072:4608], dcast2[:])
                dcast3 = hpool.tile([128, 2 * C], f32, tag="h", name="dcast3")
                nc.vector.tensor_copy(dcast3[:], X_bf[:, 0:1536])
                nc.sync.dma_start(dbg_d[:, 4608:6144], dcast3[:])

            # ---- outer LN2 + MLP ----
            hln2 = ln_transpose_outer(hres, 12, 18, eps5, "hln", hlnp)
            h_next = hpool.tile([128, 2 * C], f32, tag="h", name="h")
            # py2 banks: A=lc0[0:512], B=lc1[0:512], Cb=[lc0 512:768|lc1 512:768]
            pA = psy.tile([128, 512], f32, tag="psy", name="psy")
            pB = psy.tile([128, 512], f32, tag="psy", name="psy")
            pC1 = psy.tile([128, 512], f32, tag="psy", name="psy")
            pC2 = psy.tile([128, 512], f32, tag="psy", name="psy")
            y2tgt = [(pA, 0, 512, 0, 0), (pC1, 0, 256, 0, 512),
                     (pB, 0, 512, 1, 0), (pC2, 0, 256, 1, 512)]
            for piece in range(4):
                w1p = ww1.tile([128, 6 * C], bf, tag="ww1", name="ww1")
                nc.sync.dma_start(
                    w1p[:], dram["w1T"][li, :, piece * 4608:(piece + 1) * 4608])
                w2p = ww2.tile([128, 6 * C], bf, tag="ww2", name="ww2")
                nc.sync.dma_start(
                    w2p[:], dram["w2T"][li, :, piece * 4608:(piece + 1) * 4608])
                for fcl in range(6):
                    fc = piece * 6 + fcl
                    p1 = ps.tile([128, 512], f32, tag="ps", name="ps")
                    for cb in range(6):
                        mm(p1[:, 0:256],
                           w1p[:, fcl * C + cb * 128:fcl * C + cb * 128 + 128],
                           hln2[:, cb * 256:(cb + 1) * 256], cb == 0, cb == 5)
                    y1g = y1p.tile([128, 256], bf, tag="y1", name="y1")
                    nc.scalar.activation(y1g[:], p1[:, 0:256], AF.Gelu,
                                         bias=b1f_t[:, fc:fc + 1])
                    for pt, po, nw, lc, n0 in y2tgt:
                        mm(pt[:, po:po + nw], y1g[:, lc * 128:lc * 128 + 128],
                           w2p[:, fcl * C + n0:fcl * C + n0 + nw],
                           fc == 0, False)
            for pt, po, nw, lc, n0 in y2tgt:
                mm(pt[:, po:po + nw], ones2[0:2, 0:128],
                   rowb_t[0:2, C + n0:C + n0 + nw], False, True)
                nc.vector.tensor_add(
                    h_next[:, lc * C + n0:lc * C + n0 + nw], pt[:, po:po + nw],
                    hres[:, lc * C + n0:lc * C + n0 + nw])
            h_t = h_next

        if dbg_d is not None:
            nc.sync.dma_start(dbg_d[:, 6144:7680], h_t[:])
        if not DBG_EPI:
            logits = scr.tile([10, 1], f32, tag="logits", name="logits")
            nc.gpsimd.memset(logits[:], 0.0)
            nc.sync.dma_start(out_d[:], logits[:])
            return
        # ---------------- epilogue ----------------
        # pooled^T (FM fold [128, 6]) = mean over tokens
        pooled = scr.tile([128, 8], f32, tag="pooled", name="pooled")
        hbf = scr.tile([128, 2 * C], bf, tag="lnsub", name="hfin")
        nc.vector.tensor_copy(hbf[:], h_t[:])
        for cb in range(6):
            p = psy.tile([128, 512], f32, tag="psy", name="psy")
            for lc in range(2):
                mm(p[:, 0:1], hbf[:, lc * C + cb * 128:lc * C + cb * 128 + 128],
                   onescol[0:128, 0:1], lc == 0, lc == 1)
            nc.vector.tensor_scalar_mul(pooled[:, cb:cb + 1], p[:, 0:1],
                                        1.0 / L)
        # LN over all 768 (partition+fold): stats via f32 matmuls
        sq = scr.tile([128, 8], bf, tag="pooledsq", name="pooledsq")
        sqa = stp.tile([128, 1], f32, tag="st", name="st")
        nc.vector.scalar_tensor_tensor(
            sq[:, 0:6], pooled[:, 0:6], 1.0, pooled[:, 0:6], ALU.mult,
            ALU.mult, accum_out=sqa[:])
        sqab = stp.tile([128, 1], bf, tag="stb", name="stb")
        nc.vector.tensor_copy(sqab[:], sqa[:])
        sma = stp.tile([128, 1], f32, tag="st", name="st")
        nc.vector.reduce_sum(sma[:], pooled[:, 0:6], axis=X_AXIS)
        smab = stp.tile([128, 1], bf, tag="stb", name="stb")
        nc.vector.tensor_copy(smab[:], sma[:])
        pst = psy.tile([128, 512], f32, tag="psy", name="psy")
        mm(pst[0:1, 0:1], smab[:], onescol[0:128, 0:1], True, True)
        mm(pst[0:1, 1:2], sqab[:], onescol[0:128, 0:1], True, True)
        stat2 = zp.tile([1, 2], f32, tag="st2", name="st2")
        nc.vector.tensor_copy(stat2[:], pst[0:1, 0:2])
        mean = zp.tile([1, 2], f32, tag="mv2e", name="mv2e")
        nc.vector.tensor_scalar_mul(mean[:, 0:1], stat2[:, 0:1], 1.0 / C)
        m2 = zp.tile([1, 1], f32, tag="m2", name="m2")
        nc.vector.tensor_tensor(m2[:], mean[:, 0:1], mean[:, 0:1], ALU.mult)
        var = zp.tile([1, 1], f32, tag="var", name="var")
        nc.vector.scalar_tensor_tensor(var[:], stat2[:, 1:2], 1.0 / C, m2[:],
                                       ALU.mult, ALU.subtract)
        sde = zp.tile([1, 1], f32, tag="sde", name="sde")
        nc.scalar.activation(sde[:], var[:], AF.Sqrt, bias=eps5[0:1, :])
        rr = zp.tile([1, 1], f32, tag="rr", name="rr")
        nc.vector.reciprocal(rr[:], sde[:])
        mrb = zp.tile([2, 2], bf, tag="mrb", name="mrb")
        nc.gpsimd.memset(mrb[:], 0.0)
        nc.vector.tensor_copy(mrb[0:1, 0:1], mean[:, 0:1])
        nc.vector.tensor_copy(mrb[0:1, 1:2], rr[:])
        # broadcast mean, rstd to [128, 1] via K=2 bf16 matmul
        pbc = psy.tile([128, 512], f32, tag="psy", name="psy")
        mm(pbc[0:128, 0:2], ones2[0:2, 0:128], mrb[0:2, 0:2], True, True)
        mbc = stp.tile([128, 1], f32, tag="st", name="st")
        nc.vector.tensor_copy(mbc[:], pbc[0:128, 0:1])
        rbc = stp.tile([128, 1], f32, tag="st", name="st")
        nc.vector.tensor_copy(rbc[:], pbc[0:128, 1:2])
        pn = scr.tile([128, 8], f32, tag="pn", name="pn")
        nc.vector.tensor_scalar(pn[:, 0:6], pooled[:, 0:6], mbc[:], rbc[:],
                                ALU.subtract, ALU.mult)
        nc.vector.tensor_tensor(pn[:, 0:6], pn[:, 0:6], normgb[:, 0:6],
                                ALU.mult)
        nc.vector.tensor_add(pn[:, 0:6], pn[:, 0:6], normgb[:, 6:12])
        # head (f32 matmuls)
        ph = psy.tile([128, 512], f32, tag="psy", name="psy")
        for cb in range(6):
            mm(ph[0:10, 0:1], headwT[:, cb * 10:(cb + 1) * 10],
               pn[:, cb:cb + 1], cb == 0, cb == 5)
        logits = scr.tile([10, 1], f32, tag="logits", name="logits")
        nc.vector.tensor_add(logits[:], ph[0:10, 0:1], headb[0:10, 0:1])
        nc.sync.dma_start(out_d[:], logits[:])


_NC_CACHE = {}
TRACE = False
TRACE_TMPDIR = None
LAST = {}


def _get_nc(in_map):
    key = "k"
    if key not in _NC_CACHE:
        _NC_CACHE[key] = build(in_map)
    return _NC_CACHE[key]


def kernel(**inputs):
    per_core = marshal(inputs)
    nc = _get_nc(per_core[0])
    kw = {}
    if TRACE and TRACE_TMPDIR:
        kw["tmpdir"] = TRACE_TMPDIR
    res = run_bass_kernel_spmd(nc, per_core, core_ids=list(range(N_CORES)),
                               trace=TRACE, **kw)
    LAST["exec_time_ns"] = res.exec_time_ns
    out = np.stack([res.results[b]["out"][:, 0] for b in range(N_CORES)])
    return out.astype(np.float32)


# revision 42
# speedup vs baseline: 1.0282x; 1.0021x over previous
"""Trainium2 Bass kernel for nn_CIFARViT: 8-layer ViT with a per-head
mini-transformer over attention maps. Data-parallel: one batch element
per NeuronCore (8 cores), full inputs in / full outputs out.

v2 (engine-balance rewrite):
  - softmax max-subtraction dropped (logits verified |s| < 2).
  - all per-head [128,1] stat ops batched into multi-column tiles; one
    reciprocal per stage instead of per (head, chunk).
  - rsqrt = Exp(-0.5*Ln(v+eps)) on ACT (natural_log_exp table set, same
    set as the score exps -> no table thrash).
  - LayerNorm stats via bn_stats/bn_aggr.
  - gamma/beta + PSUM->SBUF moves fused into ACT Identity ops; plain
    copies alternate DVE/ACT to balance engine load.
  - mini-transformer restructured into 4-head stage-batched groups.
  - per-head K=96 proj (kills the 24-way oT copy split).
  - wn column-sum reciprocals: transpose Z rows to columns on PE first,
    then ONE [128,16] reciprocal (was 64 serial [1,256] recips).
"""
import sys

sys.path.insert(0, "/opt/trn_rl_repo")

import numpy as np
import ml_dtypes

import concourse.bass as bass
import concourse.mybir as mybir
import concourse.tile as tile
from concourse import bacc
from concourse.bass_utils import run_bass_kernel_spmd

BF = ml_dtypes.bfloat16
F32 = np.float32
AF = mybir.ActivationFunctionType
ALU = mybir.AluOpType
bf = mybir.dt.bfloat16
f32 = mybir.dt.float32

H = 8
HD = 96
C = 768
L = 256
D_LAYERS = 8
F = 3072

N_CORES = 8
X_AXIS = mybir.AxisListType.X
import os
DBG_LAYERS = int(os.environ.get("KLAYERS", "8"))
DBG_TAP = os.environ.get("KTAP", "") == "1"
DBG_EPI = os.environ.get("KEPI", "1") == "1"


def _fold(wt):
    """[R, Cc] with R = 128*T -> [128, T*Cc] partition fold."""
    R, Cc = wt.shape
    T = R // 128
    return np.ascontiguousarray(
        wt.reshape(T, 128, Cc).transpose(1, 0, 2).reshape(128, T * Cc)
    )


def _foldv(v):
    T = v.shape[0] // 128
    return np.ascontiguousarray(v.reshape(T, 128).T)


def marshal(inputs):
    inp = {k: np.asarray(v) for k, v in inputs.items()}
    sh = {}
    qk_l, v_l, proj_l, w1_l, w2_l, lnp_l, b1_l, rowb_l = ([] for _ in range(8))
    for i in range(D_LAYERS):
        qkvT = inp["qkv_w"][i].T.astype(BF)  # [768, 2304]
        qkf = _fold(qkvT)  # [128, 6*2304]
        img = np.zeros((128, 4 * 2304), dtype=BF)
        for hp in range(4):
            for cb in range(6):
                for hh in range(2):
                    h = hp * 2 + hh
                    base = hp * 2304 + cb * 384 + hh * 192
                    img[:, base:base + 96] = \
                        qkf[:, cb * 2304 + 96 * h: cb * 2304 + 96 * h + 96]
                    img[:, base + 96:base + 192] = \
                        qkf[:, cb * 2304 + 768 + 96 * h: cb * 2304 + 768 + 96 * h + 96]
        qk_l.append(img)
        v_l.append(_fold(np.ascontiguousarray(qkvT[:, 1536:2304])))
        # per-head proj rows: [96, 8*768]  (head h rows 96h..96h+96 of W^T)
        pw = inp["proj_w"][i].T.astype(BF)  # [768, 768]
        ph = np.zeros((96, 8 * 768), dtype=BF)
        for h in range(8):
            ph[:, h * 768:(h + 1) * 768] = pw[96 * h:96 * h + 96, :]
        proj_l.append(ph)
        w1T = inp["mlp_w1"][i].T.astype(BF)  # [768, 3072]
        w1_l.append(np.ascontiguousarray(
            w1T.reshape(6, 128, 24, 128).transpose(1, 2, 0, 3).reshape(128, 24 * 768)))
        w2_l.append(_fold(inp["mlp_w2"][i].T.astype(BF)))  # [128, 24*768]
        lnp_l.append(np.concatenate(
            [_foldv(inp[k][i].astype(F32))
             for k in ("ln1_g", "ln1_b", "ln2_g", "ln2_b")], axis=1))
        b1_l.append(_foldv(inp["mlp_b1"][i].astype(F32)))
        rb = np.zeros((2, 2 * C), dtype=BF)
        rb[0, :C] = inp["proj_b"][i].astype(BF)
        rb[0, C:] = inp["mlp_b2"][i].astype(BF)
        rowb_l.append(rb)
    sh["qk_img"] = np.stack(qk_l)
    sh["v_img"] = np.stack(v_l)
    sh["projT"] = np.stack(proj_l)
    sh["w1T"] = np.stack(w1_l)
    sh["w2T"] = np.stack(w2_l)
    sh["lnp"] = np.stack(lnp_l)
    sh["b1f"] = np.stack(b1_l)
    sh["rowb"] = np.stack(rowb_l)

    sh["mqkvT"] = _fold(inp["m_qkv_w"].T.astype(BF))   # [128, 2*768]
    sh["mprojT"] = _fold(inp["m_proj_w"].T.astype(BF))  # [128, 2*256]
    sh["mw1T"] = _fold(inp["m_mlp_w1"].T.astype(BF))   # [128, 2*1024]
    sh["mw2T"] = _fold(inp["m_mlp_w2"].T.astype(BF))   # [128, 8*256]
    sh["mlnp"] = np.concatenate(
        [_foldv(inp[k].astype(F32))
         for k in ("m_ln1_g", "m_ln1_b", "m_ln2_g", "m_ln2_b")], axis=1)
    sh["mb1f"] = _foldv(inp["m_mlp_b1"].astype(F32))   # [128, 8]
    mrb_ = np.zeros((2, 512), dtype=BF)
    mrb_[0, :256] = inp["m_proj_b"].astype(BF)
    mrb_[0, 256:] = inp["m_mlp_b2"].astype(BF)
    sh["mrowb"] = mrb_

    sh["pwT"] = np.ascontiguousarray(inp["patch_w"].reshape(C, 12).T.astype(BF))
    pos = inp["pos_emb"][0].astype(F32) + inp["patch_b"][None, :].astype(F32)
    sh["pos"] = _fold(pos)  # [128, 2*768]
    sh["normgb"] = np.concatenate(
        [_foldv(inp["norm_g"].astype(F32)), _foldv(inp["norm_b"].astype(F32))],
        axis=1)  # [128, 12]
    sh["headwT"] = _fold(inp["head_w"].T.astype(F32))  # [128, 6*10]
    sh["headb"] = inp["head_b"].astype(F32).reshape(10, 1)
    sh["ident"] = np.eye(128, dtype=BF)
    sh["onescol"] = np.ones((128, 1), dtype=BF)
    sh["ones2"] = np.ones((2, 128), dtype=BF)
    e0_ = np.zeros((2, 1), dtype=BF); e0_[0, 0] = 1.0
    sh["e0"] = e0_

    x = inp["x"].astype(F32)
    per_core = []
    for b in range(N_CORES):
        pt = (x[b].reshape(3, 16, 2, 16, 2).transpose(0, 2, 4, 1, 3)
              .reshape(12, 256).astype(BF))
        m = dict(sh)
        m["patchesT"] = np.ascontiguousarray(pt)
        per_core.append(m)
    return per_core


DT_MAP = {np.dtype(BF): bf, np.dtype(np.float32): f32}


def build(in_map):
    nc = bacc.Bacc("TRN2", target_bir_lowering=False, debug=False,
                   num_devices=N_CORES)
    dram = {k: nc.dram_tensor(k, v.shape, DT_MAP[v.dtype], kind="ExternalInput")
            for k, v in in_map.items()}
    out_d = nc.dram_tensor("out", (10, 1), f32, kind="ExternalOutput")
    dbg_d = nc.dram_tensor("dbg", (128, 10 * C), f32,
                           kind="ExternalOutput") if DBG_TAP else None
    with tile.TileContext(nc) as tc:
        _body(nc, tc, dram, out_d, dbg_d)
    nc.compile()
    return nc


def _body(nc, tc, dram, out_d, dbg_d=None):
    import contextlib
    ctx = contextlib.ExitStack()
    with ctx:
        P = lambda name, bufs=1, space="SBUF": ctx.enter_context(
            tc.tile_pool(name=name, bufs=bufs, space=space))
        cpool = P("const")

        def cload(name):
            arr = dram[name]
            t = cpool.tile(list(arr.shape), arr.dtype, tag=name, name=name)
            nc.sync.dma_start(t[:], arr[:])
            return t

        mqkvT = cload("mqkvT")
        mprojT = cload("mprojT")
        mw1T = cload("mw1T")
        mw2T = cload("mw2T")
        mlnp = cload("mlnp")
        mb1f = cload("mb1f")
        mrowb = cload("mrowb")
        ident = cload("ident")
        onescol = cload("onescol")
        ones2 = cload("ones2")
        e0 = cload("e0")
        normgb = cload("normgb")
        headwT = cload("headwT")
        headb = cload("headb")
        pwT = cload("pwT")
        patchesT = cload("patchesT")

        NDIAG = 3
        dtiles = []
        for j in range(NDIAG):
            t = cpool.tile([128, 512], bf, tag=f"diag{j}", name=f"diag{j}")
            nc.gpsimd.memset(t[:], 0.0)
            dtiles.append(t)
        dctr = [0]

        eps6 = cpool.tile([128, 1], f32, tag="eps6", name="eps6")
        nc.gpsimd.memset(eps6[:], 1e-6)
        eps5 = cpool.tile([128, 1], f32, tag="eps5", name="eps5")
        nc.gpsimd.memset(eps5[:], 1e-5)

        def build_diag(s0, s1):
            d = dtiles[dctr[0] % NDIAG]
            dctr[0] += 1
            nc.vector.tensor_scalar_mul(d[:, 0:128], ident[:], s0)
            nc.vector.tensor_scalar_mul(d[:, 384:512], ident[:], s1)
            return d

        # alternate PSUM->SBUF moves between DVE and ACT
        cctr = [0]

        def pcopy(dst, src):
            cctr[0] += 1
            if cctr[0] % 3 != 0:
                nc.vector.tensor_copy(dst, src)
            else:
                nc.scalar.activation(dst, src, AF.Copy)

        hpool = P("h", bufs=3)
        ps = P("ps", bufs=4, space="PSUM")
        psy = P("psy", bufs=4, space="PSUM")
        stp = P("st", bufs=8)
        bsp = P("bst", bufs=16)
        wqk = P("wqk", bufs=2)
        wv = P("wv", bufs=1)
        wproj = P("wproj", bufs=1)
        ww1 = P("ww1", bufs=2)
        ww2 = P("ww2", bufs=2)
        hlnp = P("hln", bufs=1)
        qkp = P("qkt", bufs=1)
        vtp = P("vt", bufs=1)
        ebp = P("eb", bufs=1)
        atp = P("at", bufs=1)
        ybp = P("yb", bufs=1)
        xbp = P("xb", bufs=1)
        scr = P("scr", bufs=2)
        scrq = P("scrq", bufs=3)
        sqp = P("sqp", bufs=2)
        gp = P("gp", bufs=1)
        y1p = P("y1", bufs=3)
        zp = P("zp", bufs=2)
        zrp = P("zrp", bufs=1)
        rbp = P("rbp", bufs=1)

        def bst(w=8):
            return bsp.tile([128, w], f32, tag=f"bst{w}", name="bst")

        def mm(out, lhsT, rhs, start, stop):
            nc.tensor.matmul(out, lhsT, rhs, start=start, stop=stop)

        # ---------------- prologue: patch embed ----------------
        h_t = hpool.tile([128, 2 * C], f32, tag="h", name="h")
        pos_t = cpool.tile([128, 2 * C], f32, tag="pos", name="pos")
        nc.sync.dma_start(pos_t[:], dram["pos"][:])
        for lc in range(2):
            for n0, nw in ((0, 512), (512, 256)):
                p = ps.tile([128, 512], f32, tag="ps", name="ps")
                mm(p[:, 0:nw], patchesT[0:12, lc * 128:lc * 128 + 128],
                   pwT[0:12, n0:n0 + nw], True, True)
                nc.vector.tensor_add(
                    h_t[:, lc * C + n0:lc * C + n0 + nw], p[:, 0:nw],
                    pos_t[:, lc * C + n0:lc * C + n0 + nw])

        # ---------------- layers ----------------
        for li in range(DBG_LAYERS):
            lnp_t = scr.tile([128, 24], f32, tag="lnp", name="lnp")
            nc.sync.dma_start(lnp_t[:], dram["lnp"][li])
            b1f_t = scr.tile([128, 24], f32, tag="b1f", name="b1f")
            nc.sync.dma_start(b1f_t[:], dram["b1f"][li])
            rowb_t = rbp.tile([2, 2 * C], bf, tag="rowb", name="rowb")
            nc.sync.dma_start(rowb_t[:], dram["rowb"][li])
            vt_w = wv.tile([128, 6 * C], bf, tag="wv", name="wv")
            nc.sync.dma_start(vt_w[:], dram["v_img"][li])
            projT = wproj.tile([96, 8 * C], bf, tag="wproj", name="wproj")
            nc.sync.dma_start(projT[:], dram["projT"][li])

            def ln_transpose_outer(src, g_col, b_col, epsv, tag, pool):
                """TM f32 [128, 2*768] -> LN'd FM bf16 [128, 6*256].
                bn_stats stats + rsqrt via gpsimd pow; gamma/beta fused
                into the PSUM->SBUF move on ACT."""
                dst = pool.tile([128, 6 * 256], bf, tag=tag)
                subs = scr.tile([128, 2 * C], bf, tag="lnsub", name="lnsub")
                rs = bst(2)
                d = dtiles[dctr[0] % NDIAG]
                dctr[0] += 1
                for lc in range(2):
                    sl = src[:, lc * C:(lc + 1) * C]
                    st3 = scrq.tile([128, 3, 6], f32, tag="st3", name="st3")
                    for sg in range(3):
                        nc.vector.bn_stats(
                            st3[:, sg, :], sl[:, sg * 256:(sg + 1) * 256])
                    mv2 = bsp.tile([128, 2], f32, tag="mv2", name="mv2")
                    nc.vector.bn_aggr(mv2[:, :], st3[:, :, :])
                    sdc = stp.tile([128, 1], f32, tag="st", name="st")
                    nc.scalar.activation(sdc[:], mv2[:, 1:2],
                                         AF.Sqrt, bias=epsv[:])
                    # per-lc recip + half-diag: lc0 transposes start while
                    # lc1 stats are still running
                    nc.vector.reciprocal(rs[:, lc:lc + 1], sdc[:])
                    nc.vector.tensor_scalar_mul(
                        d[:, lc * 384:lc * 384 + 128], ident[:],
                        rs[:, lc:lc + 1])
                    nc.vector.tensor_scalar_sub(
                        subs[:, lc * C:(lc + 1) * C], sl, mv2[:, 0:1])
                for cb in range(6):
                    p = ps.tile([128, 512], f32, tag="ps", name="ps")
                    for lc in range(2):
                        mm(p[:, lc * 128:lc * 128 + 128],
                           subs[:, lc * C + cb * 128:lc * C + cb * 128 + 128],
                           d[:, lc * 384:lc * 384 + 128], True, True)
                    nc.scalar.activation(
                        dst[:, cb * 256:(cb + 1) * 256], p[:, 0:256],
                        AF.Identity,
                        bias=lnp_t[:, b_col + cb:b_col + cb + 1],
                        scale=lnp_t[:, g_col + cb:g_col + cb + 1])
                return dst

            hln = ln_transpose_outer(h_t, 0, 6, eps6, "hln", hlnp)

            # ---- qkv: q^T|k^T per head [96, 512]; v token-major ----
            qk_bf = qkp.tile([128, 4096], bf, tag="qk", name="qk")
            for hp in range(4):
                qkw = wqk.tile([128, 2304], bf, tag="wqk", name="wqk")
                nc.sync.dma_start(
                    qkw[:], dram["qk_img"][li, :, hp * 2304:(hp + 1) * 2304])
                for hh in range(2):
                    h = hp * 2 + hh
                    p = ps.tile([128, 512], f32, tag="ps", name="ps")
                    for half in range(2):
                        for cb in range(6):
                            lh = qkw[:, cb * 384 + hh * 192 + half * 96:
                                     cb * 384 + hh * 192 + half * 96 + 96]
                            mm(p[0:96, half * 256:half * 256 + 256], lh,
                               hln[:, cb * 256:(cb + 1) * 256],
                               cb == 0, cb == 5)
                    pcopy(qk_bf[0:96, h * 512:(h + 1) * 512], p[0:96, :])

            # ---- attention scores -> E (exp, no max-sub) ----
            E_bf = ebp.tile([128, H * 512], bf, tag="eb", name="eb")
            izz = zp.tile([128, 32], f32, tag="izz", name="izz")
            for h in range(H):
                p = ps.tile([128, 512], f32, tag="ps", name="ps")
                for lc in range(2):
                    mm(p[:, lc * 256:lc * 256 + 256],
                       qk_bf[0:96, h * 512 + lc * 128:h * 512 + lc * 128 + 128],
                       qk_bf[0:96, h * 512 + 256:h * 512 + 512], True, True)
                for lc in range(2):
                    nc.scalar.activation(
                        E_bf[:, h * 512 + lc * 256:h * 512 + lc * 256 + 256],
                        p[:, lc * 256:lc * 256 + 256], AF.Exp,
                        scale=HD ** -0.5,
                        accum_out=izz[:, h * 2 + lc:h * 2 + lc + 1])
            nc.vector.reciprocal(izz[:, 16:32], izz[:, 0:16])

            # ---- v token-major (PE busy while recip runs) ----
            vt_bf = vtp.tile([128, 2 * C], bf, tag="vt", name="vt")
            for lc in range(2):
                for n0, nw in ((0, 512), (512, 256)):
                    p = ps.tile([128, 512], f32, tag="ps", name="ps")
                    for cb in range(6):
                        mm(p[:, 0:nw],
                           hln[:, cb * 256 + lc * 128:cb * 256 + lc * 128 + 128],
                           vt_w[:, cb * C + n0:cb * C + n0 + nw],
                           cb == 0, cb == 5)
                    pcopy(vt_bf[:, lc * C + n0:lc * C + n0 + nw], p[:, 0:nw])

            # ---- aT = (E/Z)^T via transpose-with-diag ----
            aT_bf = atp.tile([128, H * 512], bf, tag="at", name="at")
            for h in range(H):
                d = build_diag(izz[:, 16 + 2 * h:17 + 2 * h],
                               izz[:, 17 + 2 * h:18 + 2 * h])
                pa = ps.tile([128, 512], f32, tag="ps", name="ps")
                for mc in range(2):
                    for lc in range(2):
                        mm(pa[:, mc * 256 + lc * 128:mc * 256 + lc * 128 + 128],
                           E_bf[:, h * 512 + lc * 256 + mc * 128:
                                h * 512 + lc * 256 + mc * 128 + 128],
                           d[:, lc * 384:lc * 384 + 128], True, True)
                pcopy(aT_bf[:, h * 512:(h + 1) * 512], pa[:])

            # ---- attnV -> oT ([96, 8*256], head h at cols h*256) ----
            oT_bf = qkp.tile([128, 4096], bf, tag="qk", name="ot")
            for hq in range(4):
                po = ps.tile([128, 512], f32, tag="ps", name="ps")
                for hh in range(2):
                    h = hq * 2 + hh
                    for mc in range(2):
                        mm(po[0:96, hh * 256:hh * 256 + 256],
                           vt_bf[:, mc * C + 96 * h:mc * C + 96 * h + 96],
                           aT_bf[:, h * 512 + mc * 256:h * 512 + mc * 256 + 256],
                           mc == 0, mc == 1)
                pcopy(oT_bf[0:96, hq * 512:(hq + 1) * 512], po[0:96, :])

            # ---- proj -> y (TM bf16), per-head K=96 accumulation ----
            y_bf = ybp.tile([128, 2 * C], bf, tag="yb", name="yb")
            for lc in range(2):
                for n0, nw in ((0, 512), (512, 256)):
                    p = ps.tile([128, 512], f32, tag="ps", name="ps")
                    for h in range(H):
                        mm(p[:, 0:nw],
                           oT_bf[0:96, h * 256 + lc * 128:h * 256 + lc * 128 + 128],
                           projT[0:96, h * C + n0:h * C + n0 + nw],
                           h == 0, h == 7)
                    pcopy(y_bf[:, lc * C + n0:lc * C + n0 + nw], p[:, 0:nw])

            # ---- mini transformer: 2 groups of 4 heads, stage-batched ----
            X_bf = xbp.tile([128, H * 512], bf, tag="xb", name="xb")
            zrowb_t = zrp.tile([2, H * 256], bf, tag="zrowb", name="zrowb")
            nc.gpsimd.memset(zrowb_t[:], 0.0)
            mizz = zp.tile([128, 32], f32, tag="mizz", name="mizz")
            izc_t = zp.tile([128, 16], f32, tag="izc", name="izc")
            for g in range(2):
                h0 = 4 * g
                izg = izz[:, 16 + 8 * g:24 + 8 * g]   # 1/Z outer, 8 cols
                zg = izz[:, 8 * g:8 + 8 * g]          # Z outer
                # -- mini LN1 on a = E/Z: batched stats --
                sqE = bsp.tile([128, 8], f32, tag="sqE", name="sqE")
                for i in range(8):
                    h, lc = h0 + i // 2, i % 2
                    sq = scrq.tile([128, 256], bf, tag="sq", name="sq")
                    esl = E_bf[:, h * 512 + lc * 256:h * 512 + lc * 256 + 256]
                    nc.vector.scalar_tensor_tensor(
                        sq[:], esl, 1.0, esl, ALU.mult, ALU.mult,
                        accum_out=sqE[:, i:i + 1])
                iz2 = bst()
                nc.vector.tensor_tensor(iz2[:], izg, izg, ALU.mult)
                t1 = bst()
                nc.vector.tensor_tensor(t1[:], sqE[:], iz2[:], ALU.mult)
                var = bst()
                nc.vector.tensor_scalar(var[:], t1[:], 1.0 / L, 1.0 / (L * L),
                                        ALU.mult, ALU.subtract)
                sd8 = bst()
                nc.scalar.activation(sd8[:], var[:], AF.Sqrt, bias=eps6[:])
                ra = bst()
                nc.vector.reciprocal(ra[:], sd8[:])
                rz = bst()
                nc.vector.tensor_tensor(rz[:], ra[:], izg, ALU.mult)
                zc = bst()
                nc.vector.tensor_scalar_mul(zc[:], zg, 1.0 / L)
                esub = gp.tile([128, 2048], bf, tag="esub", name="esub")
                for i in range(8):
                    h, lc = h0 + i // 2, i % 2
                    nc.vector.tensor_scalar_sub(
                        esub[:, i * 256:(i + 1) * 256],
                        E_bf[:, h * 512 + lc * 256:h * 512 + lc * 256 + 256],
                        zc[:, i:i + 1])
                wnlnT = gp.tile([128, 2048], bf, tag="wnln", name="wnln")
                for j in range(4):
                    d = build_diag(rz[:, 2 * j:2 * j + 1], rz[:, 2 * j + 1:2 * j + 2])
                    pw_ = ps.tile([128, 512], f32, tag="ps", name="ps")
                    for mc in range(2):
                        for lc in range(2):
                            mm(pw_[:, mc * 256 + lc * 128:mc * 256 + lc * 128 + 128],
                               esub[:, j * 512 + lc * 256 + mc * 128:
                                    j * 512 + lc * 256 + mc * 128 + 128],
                               d[:, lc * 384:lc * 384 + 128], True, True)
                    for mc in range(2):
                        nc.scalar.activation(
                            wnlnT[:, j * 512 + mc * 256:j * 512 + mc * 256 + 256],
                            pw_[:, mc * 256:mc * 256 + 256], AF.Identity,
                            bias=mlnp[:, 2 + mc:3 + mc],
                            scale=mlnp[:, 0 + mc:1 + mc])
                # -- mini qkv (q^T|k^T FM per head) --
                mqk = gp.tile([128, 4096], bf, tag="mqk", name="mqk")
                for dc in range(2):
                    pq = [ps.tile([128, 512], f32, tag="ps", name="ps")
                          for _ in range(4)]
                    for half in range(2):
                        for mc in range(2):
                            for j in range(4):
                                mm(pq[j][:, half * 256:half * 256 + 256],
                                   mqkvT[:, mc * 768 + half * 256 + dc * 128:
                                         mc * 768 + half * 256 + dc * 128 + 128],
                                   wnlnT[:, j * 512 + mc * 256:
                                         j * 512 + mc * 256 + 256],
                                   mc == 0, mc == 1)
                    for j in range(4):
                        pcopy(mqk[:, j * 1024 + dc * 512:j * 1024 + dc * 512 + 512],
                              pq[j][:])
                # -- mini v (token-major) --
                mv_bf = gp.tile([128, 2048], bf, tag="mv", name="mv")
                for j in range(4):
                    pv = ps.tile([128, 512], f32, tag="ps", name="ps")
                    for lc in range(2):
                        for mc in range(2):
                            mm(pv[:, lc * 256:lc * 256 + 256],
                               wnlnT[:, j * 512 + mc * 256 + lc * 128:
                                     j * 512 + mc * 256 + lc * 128 + 128],
                               mqkvT[:, mc * 768 + 512:mc * 768 + 768],
                               mc == 0, mc == 1)
                    pcopy(mv_bf[:, j * 512:(j + 1) * 512], pv[:])
                # -- mini scores -> Em = exp (no max-sub) --
                Em = gp.tile([128, 2048], bf, tag="em", name="em")
                for j in range(4):
                    pe = ps.tile([128, 512], f32, tag="ps", name="ps")
                    for lc in range(2):
                        for dc in range(2):
                            mm(pe[:, lc * 256:lc * 256 + 256],
                               mqk[:, j * 1024 + dc * 512 + lc * 128:
                                   j * 1024 + dc * 512 + lc * 128 + 128],
                               mqk[:, j * 1024 + dc * 512 + 256:
                                   j * 1024 + dc * 512 + 512],
                               dc == 0, dc == 1)
                    for lc in range(2):
                        zcol = 8 * g + 2 * j + lc
                        nc.scalar.activation(
                            Em[:, j * 512 + lc * 256:j * 512 + lc * 256 + 256],
                            pe[:, lc * 256:lc * 256 + 256], AF.Exp,
                            scale=0.0625,
                            accum_out=mizz[:, zcol:zcol + 1])
                    # per-head reciprocal: head j's transpose can start
                    # while head j+1's exps are still on ACT
                    zc0 = 8 * g + 2 * j
                    nc.vector.reciprocal(mizz[:, 16 + zc0:18 + zc0],
                                         mizz[:, zc0:zc0 + 2])
                # -- amT = (Em/Z)^T --
                amT = gp.tile([128, 2048], bf, tag="mqk", name="amt")
                for j in range(4):
                    zcol = 16 + 8 * g + 2 * j
                    d = build_diag(mizz[:, zcol:zcol + 1],
                                   mizz[:, zcol + 1:zcol + 2])
                    pa = ps.tile([128, 512], f32, tag="ps", name="ps")
                    for mc in range(2):
                        for lc in range(2):
                            mm(pa[:, mc * 256 + lc * 128:mc * 256 + lc * 128 + 128],
                               Em[:, j * 512 + lc * 256 + mc * 128:
                                  j * 512 + lc * 256 + mc * 128 + 128],
                               d[:, lc * 384:lc * 384 + 128], True, True)
                    pcopy(amT[:, j * 512:(j + 1) * 512], pa[:])
                # -- mini attnV -> omT (FM) --
                omT = gp.tile([128, 2048], bf, tag="em", name="omt")
                for j in range(4):
                    po = ps.tile([128, 512], f32, tag="ps", name="ps")
                    for dc in range(2):
                        for mc in range(2):
                            mm(po[:, dc * 256:dc * 256 + 256],
                               mv_bf[:, j * 512 + mc * 256 + dc * 128:
                                     j * 512 + mc * 256 + dc * 128 + 128],
                               amT[:, j * 512 + mc * 256:j * 512 + mc * 256 + 256],
                               mc == 0, mc == 1)
                    pcopy(omT[:, j * 512:(j + 1) * 512], po[:])
                # -- hmini (TM f32) = omT.T@mprojT + aT.T@mprojT + pb --
                hm = gp.tile([128, 2048], f32, tag="hm", name="hm")
                for j in range(4):
                    h = h0 + j
                    for lc in range(2):
                        p = ps.tile([128, 512], f32, tag="ps", name="ps")
                        for mc in range(2):
                            mm(p[:, 0:256],
                               omT[:, j * 512 + mc * 256 + lc * 128:
                                   j * 512 + mc * 256 + lc * 128 + 128],
                               mprojT[:, mc * 256:(mc + 1) * 256],
                               mc == 0, False)
                        for mc in range(2):
                            mm(p[:, 0:256],
                               aT_bf[:, h * 512 + mc * 256 + lc * 128:
                                     h * 512 + mc * 256 + lc * 128 + 128],
                               mprojT[:, mc * 256:(mc + 1) * 256],
                               False, False)
                        mm(p[:, 0:256], ones2[0:2, 0:128],
                           mrowb[0:2, 0:256], False, True)
                        pcopy(hm[:, j * 512 + lc * 256:j * 512 + lc * 256 + 256],
                              p[:, 0:256])
                # -- mini LN2: bn stats, batched rsqrt --
                mvs2 = bsp.tile([128, 16], f32, tag="mvs2", name="mvs2")
                sd2 = bst()
                for i in range(8):
                    st6 = scrq.tile([128, 6], f32, tag="st6", name="st6")
                    nc.vector.bn_stats(
                        st6[:, :], hm[:, i * 256:(i + 1) * 256])
                    nc.vector.bn_aggr(mvs2[:, 2 * i:2 * i + 2], st6[:, :])
                    nc.scalar.activation(sd2[:, i:i + 1],
                                         mvs2[:, 2 * i + 1:2 * i + 2],
                                         AF.Sqrt, bias=eps5[:])
                r2 = bst()
                nc.vector.reciprocal(r2[:], sd2[:])
                hsub = gp.tile([128, 2048], bf, tag="esub", name="hsub")
                for i in range(8):
                    nc.vector.tensor_scalar_sub(
                        hsub[:, i * 256:(i + 1) * 256],
                        hm[:, i * 256:(i + 1) * 256], mvs2[:, 2 * i:2 * i + 1])
                mhln = gp.tile([128, 2048], bf, tag="wnln", name="mhln")
                for j in range(4):
                    d = build_diag(r2[:, 2 * j:2 * j + 1], r2[:, 2 * j + 1:2 * j + 2])
                    ph_ = ps.tile([128, 512], f32, tag="ps", name="ps")
                    for mc in range(2):
                        for lc in range(2):
                            mm(ph_[:, mc * 256 + lc * 128:mc * 256 + lc * 128 + 128],
                               hsub[:, j * 512 + lc * 256 + mc * 128:
                                    j * 512 + lc * 256 + mc * 128 + 128],
                               d[:, lc * 384:lc * 384 + 128], True, True)
                    for mc in range(2):
                        nc.scalar.activation(
                            mhln[:, j * 512 + mc * 256:j * 512 + mc * 256 + 256],
                            ph_[:, mc * 256:mc * 256 + 256], AF.Identity,
                            bias=mlnp[:, 6 + mc:7 + mc],
                            scale=mlnp[:, 4 + mc:5 + mc])
                # -- mini MLP (4 heads batched per fc; shared LDW) --
                py2 = [psy.tile([128, 512], f32, tag="psy", name="psy")
                       for _ in range(4)]
                for fc in range(8):
                    p1 = [ps.tile([128, 512], f32, tag="ps", name="ps")
                          for _ in range(4)]
                    for mc in range(2):
                        for j in range(4):
                            mm(p1[j][:, 0:256],
                               mw1T[:, mc * 1024 + fc * 128:
                                    mc * 1024 + fc * 128 + 128],
                               mhln[:, j * 512 + mc * 256:j * 512 + mc * 256 + 256],
                               mc == 0, mc == 1)
                    for j in range(4):
                        y1g = y1p.tile([128, 256], bf, tag="y1", name="y1")
                        nc.scalar.activation(y1g[:], p1[j][:, 0:256], AF.Gelu,
                                             bias=mb1f[:, fc:fc + 1])
                        for lc in range(2):
                            mm(py2[j][:, lc * 256:lc * 256 + 256],
                               y1g[:, lc * 128:lc * 128 + 128],
                               mw2T[:, fc * 256:(fc + 1) * 256],
                               fc == 0, False)
                # bias + residual -> X = exp(wn pre-softmax)
                for j in range(4):
                    h = h0 + j
                    for lc in range(2):
                        mm(py2[j][:, lc * 256:lc * 256 + 256], ones2[0:2, 0:128],
                           mrowb[0:2, 256:512], False, True)
                        wnpre = scrq.tile([128, 256], bf, tag="wnpre", name="wnpre")
                        nc.vector.tensor_add(
                            wnpre[:], py2[j][:, lc * 256:lc * 256 + 256],
                            hm[:, j * 512 + lc * 256:j * 512 + lc * 256 + 256])
                        nc.scalar.activation(
                            X_bf[:, h * 512 + lc * 256:h * 512 + lc * 256 + 256],
                            wnpre[:], AF.Exp)
                # -- column sums Z[m] of X (2 heads per PSUM bank) --
                for jp in range(2):
                    pz = psy.tile([128, 512], f32, tag="psy", name="psy")
                    for hh in range(2):
                        h = h0 + jp * 2 + hh
                        for lc in range(2):
                            mm(pz[0:1, hh * 256:hh * 256 + 256],
                               onescol[0:128, 0:1],
                               X_bf[:, h * 512 + lc * 256:h * 512 + lc * 256 + 256],
                               lc == 0, lc == 1)
                    h2 = h0 + jp * 2
                    nc.vector.tensor_copy(
                        zrowb_t[0:1, h2 * 256:h2 * 256 + 512], pz[0:1, 0:512])
            # Z-row -> Z-cols on PE, then ONE batched reciprocal
            pzT = psy.tile([128, 512], f32, tag="psy", name="psy")
            for h in range(H):
                for mc in range(2):
                    mm(pzT[0:128, h * 2 + mc:h * 2 + mc + 1],
                       zrowb_t[0:2, h * 256 + mc * 128:h * 256 + mc * 128 + 128],
                       e0[0:2, 0:1], True, True)
            zcols = zp.tile([128, 16], f32, tag="zcols", name="zcols")
            nc.vector.tensor_copy(zcols[:], pzT[0:128, 0:16])
            nc.vector.reciprocal(izc_t[:], zcols[:])

            # ---- o_new (TM) + residual -> hres ----
            hres = hpool.tile([128, 2 * C], f32, tag="h", name="h")
            for h in range(H):
                for mc in range(2):
                    p = ps.tile([128, 512], f32, tag="ps", name="ps")
                    for lc in range(2):
                        mm(p[:, 0:96],
                           X_bf[:, h * 512 + lc * 256 + mc * 128:
                                h * 512 + lc * 256 + mc * 128 + 128],
                           y_bf[:, lc * C + 96 * h:lc * C + 96 * h + 96],
                           lc == 0, False)
                    mm(p[:, 0:96],
                       zrowb_t[0:2, h * 256 + mc * 128:h * 256 + mc * 128 + 128],
                       rowb_t[0:2, 96 * h:96 * h + 96], False, True)
                    nc.vector.scalar_tensor_tensor(
                        hres[:, mc * C + 96 * h:mc * C + 96 * h + 96],
                        p[:, 0:96], izc_t[:, h * 2 + mc:h * 2 + mc + 1],
                        h_t[:, mc * C + 96 * h:mc * C + 96 * h + 96],
                        ALU.mult, ALU.add)

            if dbg_d is not None and li == DBG_LAYERS - 1:
                dcast = hpool.tile([128, 2 * C], f32, tag="h", name="dcast")
                nc.vector.tensor_copy(dcast[:], y_bf[:])
                nc.sync.dma_start(dbg_d[:, 0:1536], dcast[:])
                nc.sync.dma_start(dbg_d[:, 1536:3072], hres[:])
                dcast2 = hpool.tile([128, 2 * C], f32, tag="h", name="dcast2")
                nc.vector.tensor_copy(dcast2[:], E_bf[:, 0:1536])
                nc.sync.dma_start(dbg_d[:, 3072:4608], dcast2[:])
                dcast3 = hpool.tile([128, 2 * C], f32, tag="h", name="dcast3")
                nc.vector.tensor_copy(dcast3[:], X_bf[:, 0:1536])
                nc.sync.dma_start(dbg_d[:, 4608:6144], dcast3[:])

            # ---- outer LN2 + MLP ----
            hln2 = ln_transpose_outer(hres, 12, 18, eps5, "hln", hlnp)
            h_next = hpool.tile([128, 2 * C], f32, tag="h", name="h")
            # py2 banks: A=lc0[0:512], B=lc1[0:512], Cb=[lc0 512:768|lc1 512:768]
            pA = psy.tile([128, 512], f32, tag="psy", name="psy")
            pB = psy.tile([128, 512], f32, tag="psy", name="psy")
            pC1 = psy.tile([128, 512], f32, tag="psy", name="psy")
            pC2 = psy.tile([128, 512], f32, tag="psy", name="psy")
            y2tgt = [(pA, 0, 512, 0, 0), (pC1, 0, 256, 0, 512),
                     (pB, 0, 512, 1, 0), (pC2, 0, 256, 1, 512)]
            for piece in range(4):
                w1p = ww1.tile([128, 6 * C], bf, tag="ww1", name="ww1")
                nc.sync.dma_start(
                    w1p[:], dram["w1T"][li, :, piece * 4608:(piece + 1) * 4608])
                w2p = ww2.tile([128, 6 * C], bf, tag="ww2", name="ww2")
                nc.sync.dma_start(
                    w2p[:], dram["w2T"][li, :, piece * 4608:(piece + 1) * 4608])
                for fcl in range(6):
                    fc = piece * 6 + fcl
                    p1 = ps.tile([128, 512], f32, tag="ps", name="ps")
                    for cb in range(6):
                        mm(p1[:, 0:256],
                           w1p[:, fcl * C + cb * 128:fcl * C + cb * 128 + 128],
                           hln2[:, cb * 256:(cb + 1) * 256], cb == 0, cb == 5)
                    y1g = y1p.tile([128, 256], bf, tag="y1", name="y1")
                    nc.scalar.activation(y1g[:], p1[:, 0:256], AF.Gelu,
                                         bias=b1f_t[:, fc:fc + 1])
                    for pt, po, nw, lc, n0 in y2tgt:
                        mm(pt[:, po:po + nw], y1g[:, lc * 128:lc * 128 + 128],
                           w2p[:, fcl * C + n0:fcl * C + n0 + nw],
                           fc == 0, False)
            for pt, po, nw, lc, n0 in y2tgt:
                mm(pt[:, po:po + nw], ones2[0:2, 0:128],
                   rowb_t[0:2, C + n0:C + n0 + nw], False, True)
                nc.vector.tensor_add(
                    h_next[:, lc * C + n0:lc * C + n0 + nw], pt[:, po:po + nw],
                    hres[:, lc * C + n0:lc * C + n0 + nw])
            h_t = h_next

        if dbg_d is not None:
            nc.sync.dma_start(dbg_d[:, 6144:7680], h_t[:])
        if not DBG_EPI:
            logits = scr.tile([10, 1], f32, tag="logits", name="logits")
            nc.gpsimd.memset(logits[:], 0.0)
            nc.sync.dma_start(out_d[:], logits[:])
            return
        # ---------------- epilogue ----------------
        # pooled^T (FM fold [128, 6]) = mean over tokens
        pooled = scr.tile([128, 8], f32, tag="pooled", name="pooled")
        hbf = scr.tile([128, 2 * C], bf, tag="lnsub", name="hfin")
        nc.vector.tensor_copy(hbf[:], h_t[:])
        for cb in range(6):
            p = psy.tile([128, 512], f32, tag="psy", name="psy")
            for lc in range(2):
                mm(p[:, 0:1], hbf[:, lc * C + cb * 128:lc * C + cb * 128 + 128],
                   onescol[0:128, 0:1], lc == 0, lc == 1)
            nc.vector.tensor_scalar_mul(pooled[:, cb:cb + 1], p[:, 0:1],
                                        1.0 / L)
        # LN over all 768 (partition+fold): stats via f32 matmuls
        sq = scr.tile([128, 8], bf, tag="pooledsq", name="pooledsq")
        sqa = stp.tile([128, 1], f32, tag="st", name="st")
        nc.vector.scalar_tensor_tensor(
            sq[:, 0:6], pooled[:, 0:6], 1.0, pooled[:, 0:6], ALU.mult,
            ALU.mult, accum_out=sqa[:])
        sqab = stp.tile([128, 1], bf, tag="stb", name="stb")
        nc.vector.tensor_copy(sqab[:], sqa[:])
        sma = stp.tile([128, 1], f32, tag="st", name="st")
        nc.vector.reduce_sum(sma[:], pooled[:, 0:6], axis=X_AXIS)
        smab = stp.tile([128, 1], bf, tag="stb", name="stb")
        nc.vector.tensor_copy(smab[:], sma[:])
        pst = psy.tile([128, 512], f32, tag="psy", name="psy")
        mm(pst[0:1, 0:1], smab[:], onescol[0:128, 0:1], True, True)
        mm(pst[0:1, 1:2], sqab[:], onescol[0:128, 0:1], True, True)
        stat2 = zp.tile([1, 2], f32, tag="st2", name="st2")
        nc.vector.tensor_copy(stat2[:], pst[0:1, 0:2])
        mean = zp.tile([1, 2], f32, tag="mv2e", name="mv2e")
        nc.vector.tensor_scalar_mul(mean[:, 0:1], stat2[:, 0:1], 1.0 / C)
        m2 = zp.tile([1, 1], f32, tag="m2", name="m2")
        nc.vector.tensor_tensor(m2[:], mean[:, 0:1], mean[:, 0:1], ALU.mult)
        var = zp.tile([1, 1], f32, tag="var", name="var")
        nc.vector.scalar_tensor_tensor(var[:], stat2[:, 1:2], 1.0 / C, m2[:],
                                       ALU.mult, ALU.subtract)
        sde = zp.tile([1, 1], f32, tag="sde", name="sde")
        nc.scalar.activation(sde[:], var[:], AF.Sqrt, bias=eps5[0:1, :])
        rr = zp.tile([1, 1], f32, tag="rr", name="rr")
        nc.vector.reciprocal(rr[:], sde[:])
        mrb = zp.tile([2, 2], bf, tag="mrb", name="mrb")
        nc.gpsimd.memset(mrb[:], 0.0)
        nc.vector.tensor_copy(mrb[0:1, 0:1], mean[:, 0:1])
        nc.vector.tensor_copy(mrb[0:1, 1:2], rr[:])
        # broadcast mean, rstd to [128, 1] via K=2 bf16 matmul
        pbc = psy.tile([128, 512], f32, tag="psy", name="psy")
        mm(pbc[0:128, 0:2], ones2[0:2, 0:128], mrb[0:2, 0:2], True, True)
        mbc = stp.tile([128, 1], f32, tag="st", name="st")
        nc.vector.tensor_copy(mbc[:], pbc[0:128, 0:1])
        rbc = stp.tile([128, 1], f32, tag="st", name="st")
        nc.vector.tensor_copy(rbc[:], pbc[0:128, 1:2])
        pn = scr.tile([128, 8], f32, tag="pn", name="pn")
        nc.vector.tensor_scalar(pn[:, 0:6], pooled[:, 0:6], mbc[:], rbc[:],
                                ALU.subtract, ALU.mult)
        nc.vector.tensor_tensor(pn[:, 0:6], pn[:, 0:6], normgb[:, 0:6],
                                ALU.mult)
        nc.vector.tensor_add(pn[:, 0:6], pn[:, 0:6], normgb[:, 6:12])
        # head (f32 matmuls)
        ph = psy.tile([128, 512], f32, tag="psy", name="psy")
        for cb in range(6):
            mm(ph[0:10, 0:1], headwT[:, cb * 10:(cb + 1) * 10],
               pn[:, cb:cb + 1], cb == 0, cb == 5)
        logits = scr.tile([10, 1], f32, tag="logits", name="logits")
        nc.vector.tensor_add(logits[:], ph[0:10, 0:1], headb[0:10, 0:1])
        nc.sync.dma_start(out_d[:], logits[:])


_NC_CACHE = {}
TRACE = False
TRACE_TMPDIR = None
LAST = {}


def _get_nc(in_map):
    key = "k"
    if key not in _NC_CACHE:
        _NC_CACHE[key] = build(in_map)
    return _NC_CACHE[key]


def kernel(**inputs):
    per_core = marshal(inputs)
    nc = _get_nc(per_core[0])
    kw = {}
    if TRACE and TRACE_TMPDIR:
        kw["tmpdir"] = TRACE_TMPDIR
    res = run_bass_kernel_spmd(nc, per_core, core_ids=list(range(N_CORES)),
                               trace=TRACE, **kw)
    LAST["exec_time_ns"] = res.exec_time_ns
    out = np.stack([res.results[b]["out"][:, 0] for b in range(N_CORES)])
    return out.astype(np.float32)
